# revision 69
# baseline (speedup 1.0000x reference)
"""Trainium2 Bass kernel for MultiHeadAttention (B=4, S=2048, D=1024, H=16).

Sharding: 8 cores = 4 batches x 2 sequence-halves, no collectives. Each
core computes full k/v projections for its batch and q/attention/
out-proj/LayerNorm for its half of the sequence. A host-side column
roll of x^T makes the program identical across cores (softmax over
keys is permutation-invariant, so the k/v column order doesn't
matter): the core's q rows are always columns [0, SQ) of its x^T.

Device program (per core), all matmuls bf16 with fp32 PSUM accumulate:
  qT = Wq @ xT[:, :SQ] + bq   [D, SQ]   (d on partitions)
  kT = Wk @ xT + bk           [D, S]
  v  = x @ Wv.T + bv          [S, D]    (s on partitions), augmented
                                        with a ones column per head
  attention per head-pair m (heads 2m / 2m+1 live on partition halves
  0:64 / 64:128 of kT/qT tile m), per q-chunk n:
    scT[j]   = kT_h[:, j-chunk] . qT_h     both heads -> one 2-bank
                                           PSUM tile [128, 2, 512]
    eT[j]    = exp(scT[j] / 8)             one wide ScalarE op, bf16
    av_h[t] += eT_h[j][:, t-chunk].T @ v_aug_h[j]
        "AV-swap": eT is the stationary operand, v_aug (64 cols + ones
        column) moves, so each matmul streams only 65 columns instead
        of 512 -> half the PE time of the classic v.T @ e orientation.
        Accumulators for one head live in one PSUM bank as [4, 65]
        slices of a [128, 4, 128] tile; a full-bank zero matmul opens
        each round (clears the has_written bits + orders the start),
        then all real matmuls accumulate with start=False.
    ao_h[t]  = av_h[t][:, 0:64] * (1/av_h[t][:, 64])   per-partition
                                           scalar -> one tensor_scalar
    aot      = PE-transpose(ao)            [q, d] -> [d, q] via the
                                           identity matmul, per 128x128
  out = aoT.T @ Wo.T + bo; LayerNorm -> DRAM.

Scheduling: weights stream per head-pair so attention on pair m starts
as soon as q/k(m) and the first v d-chunk exist; the remaining
projections fill PE slack underneath the ScalarE-bound attention.
"""

import os
import sys
from contextlib import ExitStack

for _p in ("/opt/trn_rl_repo", "/root/.axon_site/_ro/trn_rl_repo"):
    if _p not in sys.path and os.path.isdir(_p):
        sys.path.insert(0, _p)

# The kernel executes through the axon jax platform; a cpu-pinned
# JAX_PLATFORMS (used for running references) would hide the NeuronCores.
# Only safe to fix before jax is first imported.
if "jax" not in sys.modules and "axon" not in os.environ.get(
        "JAX_PLATFORMS", "axon"):
    os.environ.pop("JAX_PLATFORMS")

import ml_dtypes
import numpy as np

import concourse.bacc as bacc
import concourse.mybir as mybir
import concourse.tile as tile
from concourse import library_config
from concourse.bass_utils import run_bass_kernel_spmd

BF16 = mybir.dt.bfloat16
F32 = mybir.dt.float32
AF = mybir.ActivationFunctionType
ALU = mybir.AluOpType

HD = 64  # head dim


def build_bass(S, SQ, D, H, dtype=BF16):
    """Build the per-core Bass program. S: kv seq len, SQ: q rows handled
    by this core, D: embed dim, H: total heads."""
    assert D == H * HD
    P = 128
    ET = D // P           # e (contraction) tiles; also head-pair count
    QC = min(512, SQ)     # q free-dim chunk
    QN = SQ // QC
    KC = min(512, S)      # k-proj free-dim chunk
    KN = S // KC
    VC = min(512, D)      # v/out-proj d chunk
    VN = D // VC
    HPC = VC // HD        # heads per v chunk
    MPC = HPC // 2        # head-pairs per v chunk
    SJ = S // P           # key tiles
    TQ = SQ // P          # q row tiles

    nc = bacc.Bacc("TRN2", debug=False)

    xT = nc.dram_tensor("xT", [D, S], dtype, kind="ExternalInput").ap()
    ident = nc.dram_tensor("ident", [P, P], dtype, kind="ExternalInput").ap()
    ws = {}
    for w in ("wq", "wk"):  # host-packed per head-pair: [m, p, t*d]
        ws[w] = nc.dram_tensor(w, [ET, P, ET * P], dtype,
                               kind="ExternalInput").ap()
    for w in ("wv", "wo"):
        ws[w] = nc.dram_tensor(w, [D, D], dtype, kind="ExternalInput").ap()
    bs = {
        b: nc.dram_tensor(b, [D], F32, kind="ExternalInput").ap()
        for b in ("bq", "bk", "bv", "bo", "lnw", "lnb")
    }
    out = nc.dram_tensor("out", [SQ, D], F32, kind="ExternalOutput").ap()

    with tile.TileContext(nc) as tc, ExitStack() as ctx:
        singles = ctx.enter_context(tc.tile_pool(name="singles", bufs=1))
        qkv = ctx.enter_context(tc.tile_pool(name="qkv", bufs=1))
        xp = ctx.enter_context(tc.tile_pool(name="xp", bufs=1))
        wqk = ctx.enter_context(tc.tile_pool(name="wqk", bufs=3))
        wvp = ctx.enter_context(tc.tile_pool(name="wvp", bufs=1))
        wop = ctx.enter_context(tc.tile_pool(name="wop", bufs=1))
        expp = ctx.enter_context(tc.tile_pool(name="expp", bufs=6))
        scrp = ctx.enter_context(tc.tile_pool(name="scrp", bufs=6))
        outp = ctx.enter_context(tc.tile_pool(name="outp", bufs=3))
        lnp = ctx.enter_context(tc.tile_pool(name="lnp", bufs=3))
        mmp = ctx.enter_context(tc.tile_pool(name="mm", bufs=2, space="PSUM"))
        scp = ctx.enter_context(tc.tile_pool(name="scp", bufs=4, space="PSUM"))
        avp = ctx.enter_context(tc.tile_pool(name="avp", bufs=2, space="PSUM"))

        nc.gpsimd.load_library(library_config.proxy)

        qt = qkv.tile([P, ET, SQ], dtype, tag="qt")
        kt = qkv.tile([P, ET, S], dtype, tag="kt")
        vt = qkv.tile([P, SJ, H, HD + 1], dtype, tag="vt")
        aot = qkv.tile([P, ET, SQ], dtype, tag="aot")

        def load_wqk(m):
            wqm = wqk.tile([P, ET, P], dtype, tag="wqk", name="wqm")
            nc.scalar.dma_start(wqm, ws["wq"][m].rearrange("p (t d) -> p t d", d=P))
            wkm = wqk.tile([P, ET, P], dtype, tag="wqk", name="wkm")
            nc.scalar.dma_start(wkm, ws["wk"][m].rearrange("p (t d) -> p t d", d=P))
            return wqm, wkm

        # --- startup-ordered DMA: only what the first q-proj group and
        # first scores need goes ahead of the xt half-0 tiles on each
        # queue; the remaining weights follow behind
        def load_w(which, m, name):
            w = wqk.tile([P, ET, P], dtype, tag="wqk", name=name)
            nc.scalar.dma_start(
                w, ws[which][m].rearrange("p (t d) -> p t d", d=P))
            return w

        bqk = singles.tile([P, 2 * ET], F32, tag="bqk")
        nc.sync.dma_start(bqk[:, :ET], bs["bq"].rearrange("(t p) -> p t", p=P))
        wq0 = load_w("wq", 0, "wq0")
        xt = xp.tile([P, ET, S], dtype, tag="xt")
        H2 = S // 2
        xt_engs = (nc.sync, nc.scalar)
        xrow = xT.rearrange("(t p) s -> p t s", p=P)
        for k in range(ET):
            xt_engs[k % 2].dma_start(xt[:, k, 0:H2], xrow[:, k, 0:H2])
        wk0 = load_w("wk", 0, "wk0")
        pre0 = (wq0, wk0)
        pre1 = (load_w("wq", 1, "wq1"), load_w("wk", 1, "wk1"))
        for k in range(ET):
            xt_engs[k % 2].dma_start(xt[:, k, H2:S], xrow[:, k, H2:S])

        # --- constants ---
        nc.sync.dma_start(bqk[:, ET:], bs["bk"].rearrange("(t p) -> p t", p=P))
        # free-dim bias rows, physically replicated across partitions
        # (compute engines can't read partition-step-0 APs)
        brow = {}
        for b in ("bv", "bo", "lnw", "lnb"):
            t = singles.tile([P, D], F32, tag=b)
            nc.sync.dma_start(t[0:1, :], bs[b][None, :])
            nc.gpsimd.partition_broadcast(t, t[0:1, :])
            brow[b] = t
        eps = singles.tile([P, 1], F32, tag="eps")
        nc.vector.memset(eps, 1e-5)
        nc.vector.memset(vt[:, :, :, HD:HD + 1], 1.0)
        idt = singles.tile([P, P], dtype, tag="idt")
        nc.sync.dma_start(idt, ident)
        zrow = singles.tile([P, 4], dtype, tag="zrow")
        nc.vector.memset(zrow, 0.0)

        def q_chunk(m, wqm, n):
            ps = mmp.tile([P, 512], F32, tag="mm", name="ps")[:, :QC]
            for k in range(ET):
                nc.tensor.matmul(
                    ps, wqm[:, k, :], xt[:, k, n * QC:(n + 1) * QC],
                    start=(k == 0), stop=(k == ET - 1),
                )
            nc.vector.tensor_scalar_add(
                qt[:, m, n * QC:(n + 1) * QC], ps, bqk[:, m:m + 1])

        def q_late(m):
            """q chunks beyond n=0, deferred into the n=1 sweep as PE
            filler (the wq tile is re-fetched; DMA has big slack)."""
            wqm = wqk.tile([P, ET, P], dtype, tag="wqk", name="wqm2")
            nc.scalar.dma_start(
                wqm, ws["wq"][m].rearrange("p (t d) -> p t d", d=P))
            for n in range(1, QN):
                q_chunk(m, wqm, n)

        def qk_proj(m, pre=None):
            """q (n=0 chunk) and k projections for head-pair m."""
            wqm, wkm = pre if pre is not None else load_wqk(m)
            q_chunk(m, wqm, 0)
            for n in range(KN):
                ps = mmp.tile([P, 512], F32, tag="mm", name="ps")[:, :KC]
                for k in range(ET):
                    nc.tensor.matmul(
                        ps, wkm[:, k, :], xt[:, k, n * KC:(n + 1) * KC],
                        start=(k == 0), stop=(k == ET - 1),
                    )
                nc.vector.tensor_scalar_add(
                    kt[:, m, n * KC:(n + 1) * KC], ps,
                    bqk[:, ET + m:ET + m + 1])

        def load_wv(n):
            wvn = wvp.tile([P, ET, VC], dtype, tag="wv", name="wvn")
            nc.scalar.dma_start(
                wvn,
                ws["wv"].rearrange("(t p) d -> p t d", p=P)[:, :, n * VC:(n + 1) * VC])
            return wvn

        def v_block(n, wvn, j0):
            """v projection d-chunk n, s-tiles j0..j0+1."""
            for j in range(j0, min(j0 + 2, SJ)):
                ps = mmp.tile([P, 512], F32, tag="mm", name="ps")[:, :VC]
                for k in range(ET):
                    nc.tensor.matmul(
                        ps, xt[:, k, j * P:(j + 1) * P], wvn[:, k, :],
                        start=(k == 0), stop=(k == ET - 1),
                    )
                nc.vector.tensor_tensor(
                    vt[:, j, n * HPC:(n + 1) * HPC, 0:HD],
                    ps.rearrange("p (h d) -> p h d", d=HD),
                    brow["bv"][:, n * VC:(n + 1) * VC].rearrange(
                        "p (h d) -> p h d", d=HD),
                    ALU.add,
                )

        def v_proj(n, wvn):
            for j0 in range(0, SJ, 2):
                v_block(n, wvn, j0)

        # Schraudolph exp for the DVE-offloaded score tiles: the bf16 bit
        # pattern of exp(s/8) is approximated by int16(s*A + B); the linear
        # mantissa interpolation costs ~1.8% rms on those keys, which the
        # softmax ratio mostly cancels.
        SCH_A = 0.125 * 128.0 / float(np.log(2.0))
        SCH_B = 16256.0 - 7.25

        def dve_exp(n, j):
            # the n=0 sweep is PE-bound (projection filler available), so
            # ScalarE alone keeps up; the filler-dry n=1 sweep needs the
            # exp cadence split across both engines
            return (j % 2 == 1) != (j >= 12)

        def att_exp(m, n, j, on_dve=False):
            """score pair + exp for (head pair m, q-chunk n, k-tile j).

            Per-head single-bank score tiles: exp of head h waits only on
            head h's score matmul, and the next scores' WAR on the psum
            bank clears per head — a 4-deep half-j pipeline instead of a
            2-deep full-j one."""
            et = expp.tile([P, 2, 512], dtype, tag="exp", name="et")
            for h in range(2):
                sc = scp.tile([P, 512], F32, tag="sc", name="sc")
                nc.tensor.matmul(
                    sc[:, :QC],
                    kt[h * HD:(h + 1) * HD, m, j * P:(j + 1) * P],
                    qt[h * HD:(h + 1) * HD, m, n * QC:(n + 1) * QC],
                )
                if on_dve:
                    nc.vector.tensor_scalar(
                        et.bitcast(mybir.dt.int16)[:, h, :QC], sc[:, :QC],
                        SCH_A, SCH_B, ALU.mult, ALU.add)
                else:
                    nc.scalar.activation(et[:, h, :QC], sc[:, :QC],
                                         AF.Exp, scale=0.125)
            return et

        QT = QC // P  # 128-row q subtiles per q-chunk

        def attention(m, n, deferred):
            """q-chunk n of head pair m (heads 2m, 2m+1). AV-swap: et is
            stationary, v_aug moves; accumulators av_h[:, t, 0:65] hold
            [q, av|denom] per q-subtile t, one PSUM bank per head.

            `deferred` holds the previous block's transpose emitters; they
            are woven into this block's j-loop so the in-order PE stream
            never stalls on the previous block's DVE norms at the block
            boundary. Returns this block's deferred emitters."""
            avs = [None, None]

            def emit_avs(et, j):
                for h in range(2):
                    if avs[h] is None:
                        # lazy: the zero matmul (clears the bank's
                        # has_written bits and orders, via the full-tile
                        # write, ahead of the start=False accumulation)
                        # waits on the previous block's norm reads through
                        # pool-buf reuse, so it must come after this
                        # block's first scores in the stream
                        avs[h] = avp.tile([P, QT, P], F32, tag="av",
                                          name="av")
                        # 4-element zero matmul: start=True clears the whole
                        # bank's has_written bits; writing one column of each
                        # accumulator slice orders it ahead of all 4 groups
                        nc.tensor.matmul(
                            avs[h][:, :, 0], idt, zrow,
                            start=True, stop=True, skip_group_check=True)
                    av = avs[h]
                    for t in range(QT):
                        nc.tensor.matmul(
                            av[:, t, :HD + 1],
                            et[:, h, t * P:(t + 1) * P],
                            vt[:, j, 2 * m + h, :],
                            start=False, stop=(j == SJ - 1 and t == QT - 1),
                            skip_group_check=True,
                        )

            # software pipeline: av(j-1) is emitted after score/exp(j) so the
            # in-order PE stream never stalls a score matmul behind an av
            # that waits on exp(j)
            prev = None
            for j in range(SJ):
                et = att_exp(m, n, j, on_dve=dve_exp(n, j))
                if deferred and j in (2, 4, 6, 8):
                    deferred.pop(0)()
                if prev is not None:
                    emit_avs(prev, j - 1)
                prev = et
            emit_avs(prev, SJ - 1)
            for fn in deferred:
                fn()
            # norms/evictions ride DVE in the n=0 sweep and ScalarE in the
            # filler-dry n=1 sweep, where DVE carries half the exp load
            scr = scrp.tile([P, QT, P], dtype, tag="scr", name="scr")
            for h, av in enumerate(avs):
                rcp = scrp.tile([P, QT], F32, tag="rcp", name="rcp")
                nc.vector.reciprocal(rcp, av[:, :, HD])
                # one strided op for all 4 q-subtiles; the reciprocal rides
                # a stride-0 free dim
                nc.vector.tensor_tensor(
                    scr[:, :, h * HD:(h + 1) * HD],
                    av[:, :, :HD],
                    rcp.unsqueeze(-1).broadcast_to([P, QT, HD]),
                    ALU.mult)

            def transp(t):
                def emit():
                    tp = mmp.tile([P, P], dtype, tag="mm", name="tp")
                    nc.tensor.transpose(tp, scr[:, t, :], idt)
                    dst = aot[:, m, n * QC + t * P:n * QC + (t + 1) * P]
                    if n == 0:
                        nc.vector.tensor_copy(dst, tp)
                    else:
                        nc.scalar.copy(dst, tp)
                return emit

            return [transp(t) for t in range(QT)]

        def out_ln(t, mode="mid"):
            """Out-projection + LayerNorm for q row tile t. mode: "mid"
            keeps DVE light (it carries the exp offload), "spread" moves
            tail work onto the now-idle ScalarE/GpSimd, "final" minimizes
            the serial chain that sets the kernel end time."""
            FSUB = min(512, D)
            NSUB = D // FSUB
            ot = outp.tile([P, D], F32, tag="ot", name="ot")
            for nn in range(VN):
                ps = mmp.tile([P, 512], F32, tag="mm", name="ps")[:, :VC]
                for k in range(ET):
                    nc.tensor.matmul(
                        ps, aot[:, k, t * P:(t + 1) * P],
                        wo[:, k, nn * VC:(nn + 1) * VC],
                        start=(k == 0), stop=(k == ET - 1),
                    )
                if mode in ("final", "early_tail", "tail_spread"):
                    # fused psum-read + bias add: shortest serial chain
                    nc.vector.tensor_tensor(
                        ot[:, nn * VC:(nn + 1) * VC], ps,
                        brow["bo"][:, nn * VC:(nn + 1) * VC], ALU.add)
                else:
                    # evict on ScalarE (Copy shares exp's table), bias-add
                    # on GpSimd: DVE stays free
                    nc.scalar.copy(ot[:, nn * VC:(nn + 1) * VC], ps)
                    nc.gpsimd.tensor_tensor(
                        ot[:, nn * VC:(nn + 1) * VC],
                        ot[:, nn * VC:(nn + 1) * VC],
                        brow["bo"][:, nn * VC:(nn + 1) * VC], ALU.add)
            scr = lnp.tile([P, NSUB * 6 + 6], F32, tag="scr", name="scr")
            stats = scr[:, 0:NSUB * 6].rearrange("p (s f) -> p s f", f=6)
            mv = scr[:, NSUB * 6:NSUB * 6 + 2]
            rstd = scr[:, NSUB * 6 + 2:NSUB * 6 + 3]
            sv = scr[:, NSUB * 6 + 3:NSUB * 6 + 4]
            nwt = scr[:, NSUB * 6 + 4:NSUB * 6 + 5]
            otv = ot.rearrange("p (s f) -> p s f", f=FSUB)
            for sbi in range(NSUB):
                nc.vector.bn_stats(stats[:, sbi, :], otv[:, sbi, :])
            nc.vector.bn_aggr(mv, stats)
            # rsqrt(var + eps) on DVE (quake seed + one Newton step) keeps
            # Sqrt off ScalarE: sqrt shares no activation table with exp, so
            # each interleaved use would cost two 1.3us table reloads
            I32 = mybir.dt.int32
            nc.vector.tensor_scalar(sv, mv[:, 1:2], 1e-5, None, ALU.add)
            nc.vector.tensor_scalar(rstd.bitcast(I32), sv.bitcast(I32),
                                    1, None, ALU.arith_shift_right)
            nc.vector.tensor_scalar(rstd.bitcast(I32), rstd.bitcast(I32),
                                    -1, 0x5F3759DF, ALU.mult, ALU.add)
            nc.vector.tensor_tensor(nwt, sv, rstd, ALU.mult)
            nc.vector.tensor_tensor(nwt, nwt, rstd, ALU.mult)
            nc.vector.tensor_scalar(nwt, nwt, -0.5, 1.5, ALU.mult, ALU.add)
            nc.vector.tensor_tensor(rstd, rstd, nwt, ALU.mult)
            if mode in ("final", "tail_spread"):
                # normalize on ScalarE: Identity(ot*rstd + (-mu*rstd)) —
                # both are per-partition scalars, and ScalarE is idle once
                # the exps have drained, while DVE carries the bn chains
                nmu = scr[:, NSUB * 6 + 5:NSUB * 6 + 6]
                nc.vector.tensor_scalar(nmu, mv[:, 0:1], -1.0, None,
                                        ALU.mult)
                nc.vector.tensor_tensor(nmu, nmu, rstd, ALU.mult)
                nc.scalar.activation(ot, ot, AF.Identity, bias=nmu,
                                     scale=rstd)
            else:
                nc.vector.tensor_scalar(
                    ot, ot, mv[:, 0:1], rstd, ALU.subtract, ALU.mult)
            eng = nc.vector if mode == "final" else nc.gpsimd
            eng.tensor_tensor(ot, ot, brow["lnw"], ALU.mult)
            eng.tensor_tensor(ot, ot, brow["lnb"], ALU.add)
            nc.sync.dma_start(
                out.rearrange("(t p) d -> p t d", p=P)[:, t, :], ot)

        # --- emission schedule ---
        # q/k for the first v-chunk's head pairs, then v chunk 0, then
        # alternate attention blocks with the remaining projections so the
        # scheduler can fill PE gaps under ScalarE-bound attention.
        wo = wop.tile([P, ET, D], dtype, tag="wo")
        qk_proj(0, pre0)
        qk_proj(1, pre1)
        wv0 = load_wv(0)
        v_proj(0, wv0)
        nc.scalar.dma_start(wo, ws["wo"].rearrange("(t p) d -> p t d", p=P))
        # n-outer: the n=0 sweep over all head pairs interleaves with the
        # remaining q/k/v projections; after it, aot columns [0, QC) are
        # complete, so out_ln(0..QT-1) becomes the PE filler for the
        # otherwise projection-dry n=1 sweep.
        deferred = []
        for n in range(QN):
            if n == 1:
                q_late(0)
                q_late(1)
            for m in range(ET):
                deferred = attention(m, n, deferred)
                if n == 0:
                    if 2 <= m + 1 < ET and m + 1 != 1:
                        qk_proj(m + 1)
                    for vn in range(1, VN):
                        if m + 2 == vn * MPC:
                            v_proj(vn, load_wv(vn))
                else:
                    if m + 2 < ET:
                        q_late(m + 2)
                    if m % 2 == 1:
                        out_ln((m - 1) // 2, mode="early_tail")
        for fn in deferred:
            fn()
        # tail: out-proj/LN for the last q-chunk's row tiles; the very
        # last tile gets the minimum-latency all-DVE chain
        for t in range(QT, TQ):
            out_ln(t, mode="final" if t == TQ - 1 else "tail_spread")

    nc.compile()
    return nc


# ---------------------------------------------------------------- host side

_CACHE = {}


def _get_nc(S, SQ, D, H):
    key = (S, SQ, D, H)
    if key not in _CACHE:
        _CACHE[key] = build_bass(S, SQ, D, H)
    return _CACHE[key]


def make_in_maps(x, Wq, bq, Wk, bk, Wv, bv, Wo, bo, ln_w, ln_b, n_cores=8):
    """Shard full inputs into per-core input maps (batch x seq-half)."""
    B, S, D = x.shape
    halves = n_cores // B
    SQ = S // halves
    bf = ml_dtypes.bfloat16
    ET = D // 128

    def pack_qk(W):
        # [m, p, t*128+d] = W.T[t*128+p, m*128+d]
        w4 = np.asarray(W).T.reshape(ET, 128, ET, 128)  # [t, p, m, d]
        return np.ascontiguousarray(
            w4.transpose(2, 1, 0, 3).reshape(ET, 128, ET * 128)).astype(bf)

    common = {
        "ident": np.eye(128, dtype=bf),
        "wq": pack_qk(Wq),
        "wk": pack_qk(Wk),
        "wv": np.ascontiguousarray(np.asarray(Wv).T).astype(bf),
        "wo": np.ascontiguousarray(np.asarray(Wo).T).astype(bf),
        "bq": np.asarray(bq, np.float32), "bk": np.asarray(bk, np.float32),
        "bv": np.asarray(bv, np.float32), "bo": np.asarray(bo, np.float32),
        "lnw": np.asarray(ln_w, np.float32), "lnb": np.asarray(ln_b, np.float32),
    }
    in_maps = []
    for c in range(n_cores):
        b, half = c // halves, c % halves
        xTb = np.asarray(x[b]).T.astype(bf)  # [D, S]
        if half:
            xTb = np.roll(xTb, -half * SQ, axis=1)
        in_maps.append({"xT": np.ascontiguousarray(xTb), **common})
    return in_maps, SQ


def kernel(x, Wq, bq, Wk, bk, Wv, bv, Wo, bo, ln_w, ln_b, _trace=False):
    x = np.asarray(x)
    B, S, D = x.shape
    n_cores = 8
    in_maps, SQ = make_in_maps(x, Wq, bq, Wk, bk, Wv, bv, Wo, bo, ln_w, ln_b,
                               n_cores)
    nc = _get_nc(S, SQ, D, 16)
    res = run_bass_kernel_spmd(nc, in_maps, list(range(n_cores)), trace=_trace)
    out = np.empty((B, S, D), np.float32)
    halves = n_cores // B
    for c in range(n_cores):
        b, half = c // halves, c % halves
        out[b, half * SQ:(half + 1) * SQ] = res.results[c]["out"]
    kernel.last_result = res
    return out


if __name__ == "__main__":
    nc = build_bass(512, 256, 256, 4)
    print("built ok")



# revision 70
# speedup vs baseline: 1.0013x; 1.0013x over previous
"""Trainium2 Bass kernel for MultiHeadAttention (B=4, S=2048, D=1024, H=16).

Sharding: 8 cores = 4 batches x 2 sequence-halves, no collectives. Each
core computes full k/v projections for its batch and q/attention/
out-proj/LayerNorm for its half of the sequence. A host-side column
roll of x^T makes the program identical across cores (softmax over
keys is permutation-invariant, so the k/v column order doesn't
matter): the core's q rows are always columns [0, SQ) of its x^T.

Device program (per core), all matmuls bf16 with fp32 PSUM accumulate:
  qT = Wq @ xT[:, :SQ] + bq   [D, SQ]   (d on partitions)
  kT = Wk @ xT + bk           [D, S]
  v  = x @ Wv.T + bv          [S, D]    (s on partitions), augmented
                                        with a ones column per head
  attention per head-pair m (heads 2m / 2m+1 live on partition halves
  0:64 / 64:128 of kT/qT tile m), per q-chunk n:
    scT[j]   = kT_h[:, j-chunk] . qT_h     both heads -> one 2-bank
                                           PSUM tile [128, 2, 512]
    eT[j]    = exp(scT[j] / 8)             one wide ScalarE op, bf16
    av_h[t] += eT_h[j][:, t-chunk].T @ v_aug_h[j]
        "AV-swap": eT is the stationary operand, v_aug (64 cols + ones
        column) moves, so each matmul streams only 65 columns instead
        of 512 -> half the PE time of the classic v.T @ e orientation.
        Accumulators for one head live in one PSUM bank as [4, 65]
        slices of a [128, 4, 128] tile; a full-bank zero matmul opens
        each round (clears the has_written bits + orders the start),
        then all real matmuls accumulate with start=False.
    ao_h[t]  = av_h[t][:, 0:64] * (1/av_h[t][:, 64])   per-partition
                                           scalar -> one tensor_scalar
    aot      = PE-transpose(ao)            [q, d] -> [d, q] via the
                                           identity matmul, per 128x128
  out = aoT.T @ Wo.T + bo; LayerNorm -> DRAM.

Scheduling: weights stream per head-pair so attention on pair m starts
as soon as q/k(m) and the first v d-chunk exist; the remaining
projections fill PE slack underneath the ScalarE-bound attention.
"""

import os
import sys
from contextlib import ExitStack

for _p in ("/opt/trn_rl_repo", "/root/.axon_site/_ro/trn_rl_repo"):
    if _p not in sys.path and os.path.isdir(_p):
        sys.path.insert(0, _p)

# The kernel executes through the axon jax platform; a cpu-pinned
# JAX_PLATFORMS (used for running references) would hide the NeuronCores.
# Only safe to fix before jax is first imported.
if "jax" not in sys.modules and "axon" not in os.environ.get(
        "JAX_PLATFORMS", "axon"):
    os.environ.pop("JAX_PLATFORMS")

import ml_dtypes
import numpy as np

import concourse.bacc as bacc
import concourse.mybir as mybir
import concourse.tile as tile
from concourse import library_config
from concourse.bass_utils import run_bass_kernel_spmd

BF16 = mybir.dt.bfloat16
F32 = mybir.dt.float32
AF = mybir.ActivationFunctionType
ALU = mybir.AluOpType

HD = 64  # head dim


def build_bass(S, SQ, D, H, dtype=BF16):
    """Build the per-core Bass program. S: kv seq len, SQ: q rows handled
    by this core, D: embed dim, H: total heads."""
    assert D == H * HD
    P = 128
    ET = D // P           # e (contraction) tiles; also head-pair count
    QC = min(512, SQ)     # q free-dim chunk
    QN = SQ // QC
    KC = min(512, S)      # k-proj free-dim chunk
    KN = S // KC
    VC = min(512, D)      # v/out-proj d chunk
    VN = D // VC
    HPC = VC // HD        # heads per v chunk
    MPC = HPC // 2        # head-pairs per v chunk
    SJ = S // P           # key tiles
    TQ = SQ // P          # q row tiles

    nc = bacc.Bacc("TRN2", debug=False)

    xT = nc.dram_tensor("xT", [D, S], dtype, kind="ExternalInput").ap()
    ident = nc.dram_tensor("ident", [P, P], dtype, kind="ExternalInput").ap()
    ws = {}
    for w in ("wq", "wk"):  # host-packed per head-pair: [m, p, t*d]
        ws[w] = nc.dram_tensor(w, [ET, P, ET * P], dtype,
                               kind="ExternalInput").ap()
    for w in ("wv", "wo"):
        ws[w] = nc.dram_tensor(w, [D, D], dtype, kind="ExternalInput").ap()
    bs = {
        b: nc.dram_tensor(b, [D], F32, kind="ExternalInput").ap()
        for b in ("bq", "bk", "bv", "bo", "lnw", "lnb")
    }
    out = nc.dram_tensor("out", [SQ, D], F32, kind="ExternalOutput").ap()

    with tile.TileContext(nc) as tc, ExitStack() as ctx:
        singles = ctx.enter_context(tc.tile_pool(name="singles", bufs=1))
        qkv = ctx.enter_context(tc.tile_pool(name="qkv", bufs=1))
        xp = ctx.enter_context(tc.tile_pool(name="xp", bufs=1))
        wqk = ctx.enter_context(tc.tile_pool(name="wqk", bufs=3))
        wvp = ctx.enter_context(tc.tile_pool(name="wvp", bufs=1))
        wop = ctx.enter_context(tc.tile_pool(name="wop", bufs=1))
        expp = ctx.enter_context(tc.tile_pool(name="expp", bufs=6))
        scrp = ctx.enter_context(tc.tile_pool(name="scrp", bufs=6))
        outp = ctx.enter_context(tc.tile_pool(name="outp", bufs=3))
        lnp = ctx.enter_context(tc.tile_pool(name="lnp", bufs=3))
        mmp = ctx.enter_context(tc.tile_pool(name="mm", bufs=2, space="PSUM"))
        scp = ctx.enter_context(tc.tile_pool(name="scp", bufs=4, space="PSUM"))
        avp = ctx.enter_context(tc.tile_pool(name="avp", bufs=2, space="PSUM"))

        nc.gpsimd.load_library(library_config.proxy)

        qt = qkv.tile([P, ET, SQ], dtype, tag="qt")
        kt = qkv.tile([P, ET, S], dtype, tag="kt")
        vt = qkv.tile([P, SJ, H, HD + 1], dtype, tag="vt")
        aot = qkv.tile([P, ET, SQ], dtype, tag="aot")

        def load_wqk(m):
            wqm = wqk.tile([P, ET, P], dtype, tag="wqk", name="wqm")
            nc.scalar.dma_start(wqm, ws["wq"][m].rearrange("p (t d) -> p t d", d=P))
            wkm = wqk.tile([P, ET, P], dtype, tag="wqk", name="wkm")
            nc.scalar.dma_start(wkm, ws["wk"][m].rearrange("p (t d) -> p t d", d=P))
            return wqm, wkm

        # --- startup-ordered DMA: only what the first q-proj group and
        # first scores need goes ahead of the xt half-0 tiles on each
        # queue; the remaining weights follow behind
        def load_w(which, m, name):
            w = wqk.tile([P, ET, P], dtype, tag="wqk", name=name)
            nc.scalar.dma_start(
                w, ws[which][m].rearrange("p (t d) -> p t d", d=P))
            return w

        bqk = singles.tile([P, 2 * ET], F32, tag="bqk")
        nc.sync.dma_start(bqk[:, :ET], bs["bq"].rearrange("(t p) -> p t", p=P))
        wq0 = load_w("wq", 0, "wq0")
        xt = xp.tile([P, ET, S], dtype, tag="xt")
        H2 = S // 2
        xt_engs = (nc.sync, nc.scalar)
        xrow = xT.rearrange("(t p) s -> p t s", p=P)
        for k in range(ET):
            xt_engs[k % 2].dma_start(xt[:, k, 0:H2], xrow[:, k, 0:H2])
        wk0 = load_w("wk", 0, "wk0")
        pre0 = (wq0, wk0)
        pre1 = (load_w("wq", 1, "wq1"), load_w("wk", 1, "wk1"))
        for k in range(ET):
            xt_engs[k % 2].dma_start(xt[:, k, H2:S], xrow[:, k, H2:S])

        # --- constants ---
        nc.sync.dma_start(bqk[:, ET:], bs["bk"].rearrange("(t p) -> p t", p=P))
        # free-dim bias rows, physically replicated across partitions
        # (compute engines can't read partition-step-0 APs)
        brow = {}
        for b in ("bv", "bo", "lnw", "lnb"):
            t = singles.tile([P, D], F32, tag=b)
            nc.sync.dma_start(t[0:1, :], bs[b][None, :])
            nc.gpsimd.partition_broadcast(t, t[0:1, :])
            brow[b] = t
        eps = singles.tile([P, 1], F32, tag="eps")
        nc.vector.memset(eps, 1e-5)
        nc.vector.memset(vt[:, :, :, HD:HD + 1], 1.0)
        idt = singles.tile([P, P], dtype, tag="idt")
        nc.sync.dma_start(idt, ident)
        zrow = singles.tile([P, 4], dtype, tag="zrow")
        nc.vector.memset(zrow, 0.0)

        def q_chunk(m, wqm, n):
            ps = mmp.tile([P, 512], F32, tag="mm", name="ps")[:, :QC]
            for k in range(ET):
                nc.tensor.matmul(
                    ps, wqm[:, k, :], xt[:, k, n * QC:(n + 1) * QC],
                    start=(k == 0), stop=(k == ET - 1),
                )
            nc.vector.tensor_scalar_add(
                qt[:, m, n * QC:(n + 1) * QC], ps, bqk[:, m:m + 1])

        def q_late(m):
            """q chunks beyond n=0, deferred into the n=1 sweep as PE
            filler (the wq tile is re-fetched; DMA has big slack)."""
            wqm = wqk.tile([P, ET, P], dtype, tag="wqk", name="wqm2")
            nc.scalar.dma_start(
                wqm, ws["wq"][m].rearrange("p (t d) -> p t d", d=P))
            for n in range(1, QN):
                ps = mmp.tile([P, 512], F32, tag="mm", name="ps")[:, :QC]
                for k in range(ET):
                    nc.tensor.matmul(
                        ps, wqm[:, k, :], xt[:, k, n * QC:(n + 1) * QC],
                        start=(k == 0), stop=(k == ET - 1),
                    )
                # eviction on ScalarE: per-partition bias rides the
                # activation, and back-half DVE is the busier engine
                nc.scalar.activation(
                    qt[:, m, n * QC:(n + 1) * QC], ps, AF.Identity,
                    bias=bqk[:, m:m + 1])

        def qk_proj(m, pre=None):
            """q (n=0 chunk) and k projections for head-pair m."""
            wqm, wkm = pre if pre is not None else load_wqk(m)
            q_chunk(m, wqm, 0)
            for n in range(KN):
                ps = mmp.tile([P, 512], F32, tag="mm", name="ps")[:, :KC]
                for k in range(ET):
                    nc.tensor.matmul(
                        ps, wkm[:, k, :], xt[:, k, n * KC:(n + 1) * KC],
                        start=(k == 0), stop=(k == ET - 1),
                    )
                nc.vector.tensor_scalar_add(
                    kt[:, m, n * KC:(n + 1) * KC], ps,
                    bqk[:, ET + m:ET + m + 1])

        def load_wv(n):
            wvn = wvp.tile([P, ET, VC], dtype, tag="wv", name="wvn")
            nc.scalar.dma_start(
                wvn,
                ws["wv"].rearrange("(t p) d -> p t d", p=P)[:, :, n * VC:(n + 1) * VC])
            return wvn

        def v_block(n, wvn, j0):
            """v projection d-chunk n, s-tiles j0..j0+1."""
            for j in range(j0, min(j0 + 2, SJ)):
                ps = mmp.tile([P, 512], F32, tag="mm", name="ps")[:, :VC]
                for k in range(ET):
                    nc.tensor.matmul(
                        ps, xt[:, k, j * P:(j + 1) * P], wvn[:, k, :],
                        start=(k == 0), stop=(k == ET - 1),
                    )
                nc.vector.tensor_tensor(
                    vt[:, j, n * HPC:(n + 1) * HPC, 0:HD],
                    ps.rearrange("p (h d) -> p h d", d=HD),
                    brow["bv"][:, n * VC:(n + 1) * VC].rearrange(
                        "p (h d) -> p h d", d=HD),
                    ALU.add,
                )

        def v_proj(n, wvn):
            for j0 in range(0, SJ, 2):
                v_block(n, wvn, j0)

        # Schraudolph exp for the DVE-offloaded score tiles: the bf16 bit
        # pattern of exp(s/8) is approximated by int16(s*A + B); the linear
        # mantissa interpolation costs ~1.8% rms on those keys, which the
        # softmax ratio mostly cancels.
        SCH_A = 0.125 * 128.0 / float(np.log(2.0))
        SCH_B = 16256.0 - 7.25

        def dve_exp(n, j):
            # the n=0 sweep is PE-bound (projection filler available), so
            # ScalarE alone keeps up; the filler-dry n=1 sweep needs the
            # exp cadence split across both engines
            return (j % 2 == 1) != (j >= 12)

        def att_exp(m, n, j, on_dve=False):
            """score pair + exp for (head pair m, q-chunk n, k-tile j).

            Per-head single-bank score tiles: exp of head h waits only on
            head h's score matmul, and the next scores' WAR on the psum
            bank clears per head — a 4-deep half-j pipeline instead of a
            2-deep full-j one."""
            et = expp.tile([P, 2, 512], dtype, tag="exp", name="et")
            for h in range(2):
                sc = scp.tile([P, 512], F32, tag="sc", name="sc")
                nc.tensor.matmul(
                    sc[:, :QC],
                    kt[h * HD:(h + 1) * HD, m, j * P:(j + 1) * P],
                    qt[h * HD:(h + 1) * HD, m, n * QC:(n + 1) * QC],
                )
                if on_dve:
                    nc.vector.tensor_scalar(
                        et.bitcast(mybir.dt.int16)[:, h, :QC], sc[:, :QC],
                        SCH_A, SCH_B, ALU.mult, ALU.add)
                else:
                    nc.scalar.activation(et[:, h, :QC], sc[:, :QC],
                                         AF.Exp, scale=0.125)
            return et

        QT = QC // P  # 128-row q subtiles per q-chunk

        def attention(m, n, deferred):
            """q-chunk n of head pair m (heads 2m, 2m+1). AV-swap: et is
            stationary, v_aug moves; accumulators av_h[:, t, 0:65] hold
            [q, av|denom] per q-subtile t, one PSUM bank per head.

            `deferred` holds the previous block's transpose emitters; they
            are woven into this block's j-loop so the in-order PE stream
            never stalls on the previous block's DVE norms at the block
            boundary. Returns this block's deferred emitters."""
            avs = [None, None]

            def emit_avs(et, j):
                for h in range(2):
                    if avs[h] is None:
                        # lazy: the zero matmul (clears the bank's
                        # has_written bits and orders, via the full-tile
                        # write, ahead of the start=False accumulation)
                        # waits on the previous block's norm reads through
                        # pool-buf reuse, so it must come after this
                        # block's first scores in the stream
                        avs[h] = avp.tile([P, QT, P], F32, tag="av",
                                          name="av")
                        # 4-element zero matmul: start=True clears the whole
                        # bank's has_written bits; writing one column of each
                        # accumulator slice orders it ahead of all 4 groups
                        nc.tensor.matmul(
                            avs[h][:, :, 0], idt, zrow,
                            start=True, stop=True, skip_group_check=True)
                    av = avs[h]
                    for t in range(QT):
                        nc.tensor.matmul(
                            av[:, t, :HD + 1],
                            et[:, h, t * P:(t + 1) * P],
                            vt[:, j, 2 * m + h, :],
                            start=False, stop=(j == SJ - 1 and t == QT - 1),
                            skip_group_check=True,
                        )

            # software pipeline: av(j-1) is emitted after score/exp(j) so the
            # in-order PE stream never stalls a score matmul behind an av
            # that waits on exp(j)
            prev = None
            for j in range(SJ):
                et = att_exp(m, n, j, on_dve=dve_exp(n, j))
                if deferred and j in (2, 4, 6, 8):
                    deferred.pop(0)()
                if prev is not None:
                    emit_avs(prev, j - 1)
                prev = et
            emit_avs(prev, SJ - 1)
            for fn in deferred:
                fn()
            # norms/evictions ride DVE in the n=0 sweep and ScalarE in the
            # filler-dry n=1 sweep, where DVE carries half the exp load
            scr = scrp.tile([P, QT, P], dtype, tag="scr", name="scr")
            for h, av in enumerate(avs):
                rcp = scrp.tile([P, QT], F32, tag="rcp", name="rcp")
                nc.vector.reciprocal(rcp, av[:, :, HD])
                # one strided op for all 4 q-subtiles; the reciprocal rides
                # a stride-0 free dim
                nc.vector.tensor_tensor(
                    scr[:, :, h * HD:(h + 1) * HD],
                    av[:, :, :HD],
                    rcp.unsqueeze(-1).broadcast_to([P, QT, HD]),
                    ALU.mult)

            def transp(t):
                def emit():
                    tp = mmp.tile([P, P], dtype, tag="mm", name="tp")
                    nc.tensor.transpose(tp, scr[:, t, :], idt)
                    dst = aot[:, m, n * QC + t * P:n * QC + (t + 1) * P]
                    if n == 0:
                        nc.vector.tensor_copy(dst, tp)
                    else:
                        nc.scalar.copy(dst, tp)
                return emit

            return [transp(t) for t in range(QT)]

        def out_ln(t, mode="mid"):
            """Out-projection + LayerNorm for q row tile t. mode: "mid"
            keeps DVE light (it carries the exp offload), "spread" moves
            tail work onto the now-idle ScalarE/GpSimd, "final" minimizes
            the serial chain that sets the kernel end time."""
            FSUB = min(512, D)
            NSUB = D // FSUB
            ot = outp.tile([P, D], F32, tag="ot", name="ot")
            for nn in range(VN):
                ps = mmp.tile([P, 512], F32, tag="mm", name="ps")[:, :VC]
                for k in range(ET):
                    nc.tensor.matmul(
                        ps, aot[:, k, t * P:(t + 1) * P],
                        wo[:, k, nn * VC:(nn + 1) * VC],
                        start=(k == 0), stop=(k == ET - 1),
                    )
                if mode in ("final", "early_tail"):
                    # fused psum-read + bias add: shortest serial chain
                    nc.vector.tensor_tensor(
                        ot[:, nn * VC:(nn + 1) * VC], ps,
                        brow["bo"][:, nn * VC:(nn + 1) * VC], ALU.add)
                else:
                    # evict on ScalarE (Copy shares exp's table), bias-add
                    # on GpSimd: DVE stays free
                    nc.scalar.copy(ot[:, nn * VC:(nn + 1) * VC], ps)
                    nc.gpsimd.tensor_tensor(
                        ot[:, nn * VC:(nn + 1) * VC],
                        ot[:, nn * VC:(nn + 1) * VC],
                        brow["bo"][:, nn * VC:(nn + 1) * VC], ALU.add)
            scr = lnp.tile([P, NSUB * 6 + 5], F32, tag="scr", name="scr")
            stats = scr[:, 0:NSUB * 6].rearrange("p (s f) -> p s f", f=6)
            mv = scr[:, NSUB * 6:NSUB * 6 + 2]
            rstd = scr[:, NSUB * 6 + 2:NSUB * 6 + 3]
            sv = scr[:, NSUB * 6 + 3:NSUB * 6 + 4]
            nwt = scr[:, NSUB * 6 + 4:NSUB * 6 + 5]
            otv = ot.rearrange("p (s f) -> p s f", f=FSUB)
            for sbi in range(NSUB):
                nc.vector.bn_stats(stats[:, sbi, :], otv[:, sbi, :])
            nc.vector.bn_aggr(mv, stats)
            # rsqrt(var + eps) on DVE (quake seed + one Newton step) keeps
            # Sqrt off ScalarE: sqrt shares no activation table with exp, so
            # each interleaved use would cost two 1.3us table reloads
            I32 = mybir.dt.int32
            nc.vector.tensor_scalar(sv, mv[:, 1:2], 1e-5, None, ALU.add)
            nc.vector.tensor_scalar(rstd.bitcast(I32), sv.bitcast(I32),
                                    1, None, ALU.arith_shift_right)
            nc.vector.tensor_scalar(rstd.bitcast(I32), rstd.bitcast(I32),
                                    -1, 0x5F3759DF, ALU.mult, ALU.add)
            nc.vector.tensor_tensor(nwt, sv, rstd, ALU.mult)
            nc.vector.tensor_tensor(nwt, nwt, rstd, ALU.mult)
            nc.vector.tensor_scalar(nwt, nwt, -0.5, 1.5, ALU.mult, ALU.add)
            nc.vector.tensor_tensor(rstd, rstd, nwt, ALU.mult)
            nc.vector.tensor_scalar(
                ot, ot, mv[:, 0:1], rstd, ALU.subtract, ALU.mult)
            eng = nc.vector if mode == "final" else nc.gpsimd
            eng.tensor_tensor(ot, ot, brow["lnw"], ALU.mult)
            eng.tensor_tensor(ot, ot, brow["lnb"], ALU.add)
            nc.sync.dma_start(
                out.rearrange("(t p) d -> p t d", p=P)[:, t, :], ot)

        # --- emission schedule ---
        # q/k for the first v-chunk's head pairs, then v chunk 0, then
        # alternate attention blocks with the remaining projections so the
        # scheduler can fill PE gaps under ScalarE-bound attention.
        wo = wop.tile([P, ET, D], dtype, tag="wo")
        qk_proj(0, pre0)
        qk_proj(1, pre1)
        wv0 = load_wv(0)
        v_proj(0, wv0)
        nc.scalar.dma_start(wo, ws["wo"].rearrange("(t p) d -> p t d", p=P))
        # n-outer: the n=0 sweep over all head pairs interleaves with the
        # remaining q/k/v projections; after it, aot columns [0, QC) are
        # complete, so out_ln(0..QT-1) becomes the PE filler for the
        # otherwise projection-dry n=1 sweep.
        deferred = []
        for n in range(QN):
            if n == 1:
                q_late(0)
                q_late(1)
            for m in range(ET):
                deferred = attention(m, n, deferred)
                if n == 0:
                    if 2 <= m + 1 < ET and m + 1 != 1:
                        qk_proj(m + 1)
                    for vn in range(1, VN):
                        if m + 2 == vn * MPC:
                            v_proj(vn, load_wv(vn))
                else:
                    if m + 2 < ET:
                        q_late(m + 2)
                    if m % 2 == 1:
                        out_ln((m - 1) // 2, mode="early_tail")
        for fn in deferred:
            fn()
        # tail: out-proj/LN for the last q-chunk's row tiles; the very
        # last tile gets the minimum-latency all-DVE chain
        for t in range(QT, TQ):
            out_ln(t, mode="final" if t == TQ - 1 else "early_tail")

    nc.compile()
    return nc


# ---------------------------------------------------------------- host side

_CACHE = {}


def _get_nc(S, SQ, D, H):
    key = (S, SQ, D, H)
    if key not in _CACHE:
        _CACHE[key] = build_bass(S, SQ, D, H)
    return _CACHE[key]


def make_in_maps(x, Wq, bq, Wk, bk, Wv, bv, Wo, bo, ln_w, ln_b, n_cores=8):
    """Shard full inputs into per-core input maps (batch x seq-half)."""
    B, S, D = x.shape
    halves = n_cores // B
    SQ = S // halves
    bf = ml_dtypes.bfloat16
    ET = D // 128

    def pack_qk(W):
        # [m, p, t*128+d] = W.T[t*128+p, m*128+d]
        w4 = np.asarray(W).T.reshape(ET, 128, ET, 128)  # [t, p, m, d]
        return np.ascontiguousarray(
            w4.transpose(2, 1, 0, 3).reshape(ET, 128, ET * 128)).astype(bf)

    common = {
        "ident": np.eye(128, dtype=bf),
        "wq": pack_qk(Wq),
        "wk": pack_qk(Wk),
        "wv": np.ascontiguousarray(np.asarray(Wv).T).astype(bf),
        "wo": np.ascontiguousarray(np.asarray(Wo).T).astype(bf),
        "bq": np.asarray(bq, np.float32), "bk": np.asarray(bk, np.float32),
        "bv": np.asarray(bv, np.float32), "bo": np.asarray(bo, np.float32),
        "lnw": np.asarray(ln_w, np.float32), "lnb": np.asarray(ln_b, np.float32),
    }
    in_maps = []
    for c in range(n_cores):
        b, half = c // halves, c % halves
        xTb = np.asarray(x[b]).T.astype(bf)  # [D, S]
        if half:
            xTb = np.roll(xTb, -half * SQ, axis=1)
        in_maps.append({"xT": np.ascontiguousarray(xTb), **common})
    return in_maps, SQ


def kernel(x, Wq, bq, Wk, bk, Wv, bv, Wo, bo, ln_w, ln_b, _trace=False):
    x = np.asarray(x)
    B, S, D = x.shape
    n_cores = 8
    in_maps, SQ = make_in_maps(x, Wq, bq, Wk, bk, Wv, bv, Wo, bo, ln_w, ln_b,
                               n_cores)
    nc = _get_nc(S, SQ, D, 16)
    res = run_bass_kernel_spmd(nc, in_maps, list(range(n_cores)), trace=_trace)
    out = np.empty((B, S, D), np.float32)
    halves = n_cores // B
    for c in range(n_cores):
        b, half = c // halves, c % halves
        out[b, half * SQ:(half + 1) * SQ] = res.results[c]["out"]
    kernel.last_result = res
    return out


if __name__ == "__main__":
    nc = build_bass(512, 256, 256, 4)
    print("built ok")



# revision 79
# speedup vs baseline: 1.0136x; 1.0123x over previous
"""Trainium2 Bass kernel for MultiHeadAttention (B=4, S=2048, D=1024, H=16).

Sharding: 8 cores = 4 batches x 2 sequence-halves, no collectives. Each
core computes full k/v projections for its batch and q/attention/
out-proj/LayerNorm for its half of the sequence. A host-side column
roll of x^T makes the program identical across cores (softmax over
keys is permutation-invariant, so the k/v column order doesn't
matter): the core's q rows are always columns [0, SQ) of its x^T.

Device program (per core), all matmuls bf16 with fp32 PSUM accumulate:
  qT = Wq @ xT[:, :SQ] + bq   [D, SQ]   (d on partitions)
  kT = Wk @ xT + bk           [D, S]
  v  = x @ Wv.T + bv          [S, D]    (s on partitions), augmented
                                        with a ones column per head
  attention per head-pair m (heads 2m / 2m+1 live on partition halves
  0:64 / 64:128 of kT/qT tile m), per q-chunk n:
    scT[j,h] = kT_h[:, j-chunk] . qT_h     per-head single-bank PSUM
                                           tiles (4-buf rotation) so the
                                           exp->score WAR pipelines at
                                           half-j granularity
    eT[j]    = exp(scT[j] / 8)             per-head ScalarE ops; half the
                                           j-tiles instead go through a
                                           DVE Schraudolph bit-trick
                                           (int16(A*s+B) viewed as bf16,
                                           ~1.8% rms on those keys)
    av_h[t] += eT_h[j][:, t-chunk].T @ v_aug_h[j]
        "AV-swap": eT is the stationary operand, v_aug (64 cols + ones
        column) moves, so each matmul streams only 65 columns instead
        of 512 -> half the PE time of the classic v.T @ e orientation.
        Accumulators for one head live in one PSUM bank as [4, 65]
        slices of a [128, 4, 128] tile; a tiny full-bank zero matmul
        opens each round (clears the has_written bits + orders the
        start), then all real matmuls accumulate with start=False.
    ao_h[t]  = av_h[t][:, 0:64] * (1/av_h[t][:, 64])   per-partition
                                           scalar -> one strided DVE op
    aot      = PE-transpose(ao)            [q, d] -> [d, q] via the
                                           identity matmul, per 128x128
  out = aoT.T @ Wo.T + bo; LayerNorm -> DRAM (rsqrt via DVE quake
  seed + Newton so Sqrt never evicts ScalarE's exp table).

Scheduling: n-outer sweeps; the n=0 sweep interleaves with the k/v and
n=0 q projections, the n=1 sweep is fed by the deferred n=1 q chunks
and the first half's out-proj/LayerNorm tiles. av(j-1) is emitted
after score/exp(j) and each block's transposes are woven into the next
block's j-loop so the in-order PE stream never head-of-line blocks.
"""

import os
import sys
from contextlib import ExitStack

for _p in ("/opt/trn_rl_repo", "/root/.axon_site/_ro/trn_rl_repo"):
    if _p not in sys.path and os.path.isdir(_p):
        sys.path.insert(0, _p)

# The kernel executes through the axon jax platform; a cpu-pinned
# JAX_PLATFORMS (used for running references) would hide the NeuronCores.
# Only safe to fix before jax is first imported.
if "jax" not in sys.modules and "axon" not in os.environ.get(
        "JAX_PLATFORMS", "axon"):
    os.environ.pop("JAX_PLATFORMS")

import ml_dtypes
import numpy as np

import concourse.bacc as bacc
import concourse.mybir as mybir
import concourse.tile as tile
from concourse import library_config
from concourse.bass_utils import run_bass_kernel_spmd

BF16 = mybir.dt.bfloat16
F32 = mybir.dt.float32
AF = mybir.ActivationFunctionType
ALU = mybir.AluOpType

HD = 64  # head dim


def build_bass(S, SQ, D, H, dtype=BF16):
    """Build the per-core Bass program. S: kv seq len, SQ: q rows handled
    by this core, D: embed dim, H: total heads."""
    assert D == H * HD
    P = 128
    ET = D // P           # e (contraction) tiles; also head-pair count
    QC = min(512, SQ)     # q free-dim chunk
    QN = SQ // QC
    KC = min(512, S)      # k-proj free-dim chunk
    KN = S // KC
    VC = min(512, D)      # v/out-proj d chunk
    VN = D // VC
    HPC = VC // HD        # heads per v chunk
    MPC = HPC // 2        # head-pairs per v chunk
    SJ = S // P           # key tiles
    TQ = SQ // P          # q row tiles

    nc = bacc.Bacc("TRN2", debug=False)

    xT = nc.dram_tensor("xT", [D, S], dtype, kind="ExternalInput").ap()
    ident = nc.dram_tensor("ident", [P, P], dtype, kind="ExternalInput").ap()
    ws = {}
    for w in ("wq", "wk"):  # host-packed per head-pair: [m, p, t*d]
        ws[w] = nc.dram_tensor(w, [ET, P, ET * P], dtype,
                               kind="ExternalInput").ap()
    for w in ("wv", "wo"):
        ws[w] = nc.dram_tensor(w, [D, D], dtype, kind="ExternalInput").ap()
    bs = {
        b: nc.dram_tensor(b, [D], F32, kind="ExternalInput").ap()
        for b in ("bq", "bk", "bv", "bo", "lnw", "lnb")
    }
    out = nc.dram_tensor("out", [SQ, D], F32, kind="ExternalOutput").ap()

    with tile.TileContext(nc) as tc, ExitStack() as ctx:
        singles = ctx.enter_context(tc.tile_pool(name="singles", bufs=1))
        qkv = ctx.enter_context(tc.tile_pool(name="qkv", bufs=1))
        xp = ctx.enter_context(tc.tile_pool(name="xp", bufs=1))
        wqk = ctx.enter_context(tc.tile_pool(name="wqk", bufs=3))
        wvp = ctx.enter_context(tc.tile_pool(name="wvp", bufs=1))
        wop = ctx.enter_context(tc.tile_pool(name="wop", bufs=1))
        expp = ctx.enter_context(tc.tile_pool(name="expp", bufs=6))
        scrp = ctx.enter_context(tc.tile_pool(name="scrp", bufs=6))
        outp = ctx.enter_context(tc.tile_pool(name="outp", bufs=3))
        lnp = ctx.enter_context(tc.tile_pool(name="lnp", bufs=3))
        mmp = ctx.enter_context(tc.tile_pool(name="mm", bufs=2, space="PSUM"))
        scp = ctx.enter_context(tc.tile_pool(name="scp", bufs=4, space="PSUM"))
        avp = ctx.enter_context(tc.tile_pool(name="avp", bufs=2, space="PSUM"))

        nc.gpsimd.load_library(library_config.proxy)

        qt = qkv.tile([P, ET, SQ], dtype, tag="qt")
        kt = qkv.tile([P, ET, S], dtype, tag="kt")
        vt = qkv.tile([P, SJ, H, HD + 1], dtype, tag="vt")
        aot = qkv.tile([P, ET, SQ], dtype, tag="aot")

        def load_wqk(m):
            wqm = wqk.tile([P, ET, P], dtype, tag="wqk", name="wqm")
            nc.scalar.dma_start(wqm, ws["wq"][m].rearrange("p (t d) -> p t d", d=P))
            wkm = wqk.tile([P, ET, P], dtype, tag="wqk", name="wkm")
            nc.scalar.dma_start(wkm, ws["wk"][m].rearrange("p (t d) -> p t d", d=P))
            return wqm, wkm

        # --- startup-ordered DMA: only what the first q-proj group and
        # first scores need goes ahead of the xt half-0 tiles on each
        # queue; the remaining weights follow behind
        def load_w(which, m, name):
            w = wqk.tile([P, ET, P], dtype, tag="wqk", name=name)
            nc.scalar.dma_start(
                w, ws[which][m].rearrange("p (t d) -> p t d", d=P))
            return w

        bqk = singles.tile([P, 2 * ET], F32, tag="bqk")
        nc.sync.dma_start(bqk[:, :ET], bs["bq"].rearrange("(t p) -> p t", p=P))
        wq0 = load_w("wq", 0, "wq0")
        xt = xp.tile([P, ET, S], dtype, tag="xt")
        H2 = S // 2
        xt_engs = (nc.sync, nc.scalar)
        xrow = xT.rearrange("(t p) s -> p t s", p=P)
        for k in range(ET):
            xt_engs[k % 2].dma_start(xt[:, k, 0:H2], xrow[:, k, 0:H2])
        wk0 = load_w("wk", 0, "wk0")
        pre0 = (wq0, wk0)
        pre1 = (load_w("wq", 1, "wq1"), load_w("wk", 1, "wk1"))
        for k in range(ET):
            xt_engs[k % 2].dma_start(xt[:, k, H2:S], xrow[:, k, H2:S])

        # --- constants ---
        nc.sync.dma_start(bqk[:, ET:], bs["bk"].rearrange("(t p) -> p t", p=P))
        # free-dim bias rows, physically replicated across partitions
        # (compute engines can't read partition-step-0 APs)
        brow = {}
        for b in ("bv", "bo", "lnw", "lnb"):
            t = singles.tile([P, D], F32, tag=b)
            nc.sync.dma_start(t[0:1, :], bs[b][None, :])
            nc.gpsimd.partition_broadcast(t, t[0:1, :])
            brow[b] = t
        eps = singles.tile([P, 1], F32, tag="eps")
        nc.vector.memset(eps, 1e-5)
        nc.vector.memset(vt[:, :, :, HD:HD + 1], 1.0)
        idt = singles.tile([P, P], dtype, tag="idt")
        nc.sync.dma_start(idt, ident)
        zrow = singles.tile([P, 4], dtype, tag="zrow")
        nc.vector.memset(zrow, 0.0)

        def q_chunk(m, wqm, n):
            ps = mmp.tile([P, 512], F32, tag="mm", name="ps")[:, :QC]
            for k in range(ET):
                nc.tensor.matmul(
                    ps, wqm[:, k, :], xt[:, k, n * QC:(n + 1) * QC],
                    start=(k == 0), stop=(k == ET - 1),
                )
            nc.vector.tensor_scalar_add(
                qt[:, m, n * QC:(n + 1) * QC], ps, bqk[:, m:m + 1])

        def q_late(m):
            """q chunks beyond n=0, deferred into the n=1 sweep as PE
            filler (the wq tile is re-fetched; DMA has big slack)."""
            wqm = wqk.tile([P, ET, P], dtype, tag="wqk", name="wqm2")
            nc.scalar.dma_start(
                wqm, ws["wq"][m].rearrange("p (t d) -> p t d", d=P))
            for n in range(1, QN):
                ps = mmp.tile([P, 512], F32, tag="mm", name="ps")[:, :QC]
                for k in range(ET):
                    nc.tensor.matmul(
                        ps, wqm[:, k, :], xt[:, k, n * QC:(n + 1) * QC],
                        start=(k == 0), stop=(k == ET - 1),
                    )
                nc.scalar.activation(
                    qt[:, m, n * QC:(n + 1) * QC], ps, AF.Identity,
                    bias=bqk[:, m:m + 1])

        def qk_proj(m, pre=None):
            """q (n=0 chunk) and k projections for head-pair m."""
            wqm, wkm = pre if pre is not None else load_wqk(m)
            q_chunk(m, wqm, 0)
            for n in range(KN):
                ps = mmp.tile([P, 512], F32, tag="mm", name="ps")[:, :KC]
                for k in range(ET):
                    nc.tensor.matmul(
                        ps, wkm[:, k, :], xt[:, k, n * KC:(n + 1) * KC],
                        start=(k == 0), stop=(k == ET - 1),
                    )
                nc.vector.tensor_scalar_add(
                    kt[:, m, n * KC:(n + 1) * KC], ps,
                    bqk[:, ET + m:ET + m + 1])

        def load_wv(n):
            wvn = wvp.tile([P, ET, VC], dtype, tag="wv", name="wvn")
            nc.scalar.dma_start(
                wvn,
                ws["wv"].rearrange("(t p) d -> p t d", p=P)[:, :, n * VC:(n + 1) * VC])
            return wvn

        def v_block(n, wvn, j0):
            """v projection d-chunk n, s-tiles j0..j0+1."""
            for j in range(j0, min(j0 + 2, SJ)):
                ps = mmp.tile([P, 512], F32, tag="mm", name="ps")[:, :VC]
                for k in range(ET):
                    nc.tensor.matmul(
                        ps, xt[:, k, j * P:(j + 1) * P], wvn[:, k, :],
                        start=(k == 0), stop=(k == ET - 1),
                    )
                nc.vector.tensor_tensor(
                    vt[:, j, n * HPC:(n + 1) * HPC, 0:HD],
                    ps.rearrange("p (h d) -> p h d", d=HD),
                    brow["bv"][:, n * VC:(n + 1) * VC].rearrange(
                        "p (h d) -> p h d", d=HD),
                    ALU.add,
                )

        def v_proj(n, wvn):
            for j0 in range(0, SJ, 2):
                v_block(n, wvn, j0)

        # Schraudolph exp for the DVE-offloaded score tiles: the bf16 bit
        # pattern of exp(s/8) is approximated by int16(s*A + B); the linear
        # mantissa interpolation costs ~1.8% rms on those keys, which the
        # softmax ratio mostly cancels.
        SCH_A = 0.125 * 128.0 / float(np.log(2.0))
        SCH_B = 16256.0 - 7.25

        def dve_exp(n, j):
            # the n=0 sweep is PE-bound (projection filler available), so
            # ScalarE alone keeps up; the filler-dry n=1 sweep needs the
            # exp cadence split across both engines
            return (j % 2 == 1) != (j >= 14)

        def att_exp(m, n, j, on_dve=False):
            """score pair + exp for (head pair m, q-chunk n, k-tile j).

            Per-head single-bank score tiles: exp of head h waits only on
            head h's score matmul, and the next scores' WAR on the psum
            bank clears per head — a 4-deep half-j pipeline instead of a
            2-deep full-j one."""
            et = expp.tile([P, 2, 512], dtype, tag="exp", name="et")
            for h in range(2):
                sc = scp.tile([P, 512], F32, tag="sc", name="sc")
                nc.tensor.matmul(
                    sc[:, :QC],
                    kt[h * HD:(h + 1) * HD, m, j * P:(j + 1) * P],
                    qt[h * HD:(h + 1) * HD, m, n * QC:(n + 1) * QC],
                )
                if on_dve:
                    nc.vector.tensor_scalar(
                        et.bitcast(mybir.dt.int16)[:, h, :QC], sc[:, :QC],
                        SCH_A, SCH_B, ALU.mult, ALU.add)
                else:
                    nc.scalar.activation(et[:, h, :QC], sc[:, :QC],
                                         AF.Exp, scale=0.125)
            return et

        QT = QC // P  # 128-row q subtiles per q-chunk

        def attention(m, n, deferred):
            """q-chunk n of head pair m (heads 2m, 2m+1). AV-swap: et is
            stationary, v_aug moves; accumulators av_h[:, t, 0:65] hold
            [q, av|denom] per q-subtile t, one PSUM bank per head.

            `deferred` holds the previous block's transpose emitters; they
            are woven into this block's j-loop so the in-order PE stream
            never stalls on the previous block's DVE norms at the block
            boundary. Returns this block's deferred emitters."""
            avs = [None, None]

            def emit_avs(et, j):
                for h in range(2):
                    if avs[h] is None:
                        # lazy: the zero matmul (clears the bank's
                        # has_written bits and orders, via the full-tile
                        # write, ahead of the start=False accumulation)
                        # waits on the previous block's norm reads through
                        # pool-buf reuse, so it must come after this
                        # block's first scores in the stream
                        avs[h] = avp.tile([P, QT, P], F32, tag="av",
                                          name="av")
                        # 4-element zero matmul: start=True clears the whole
                        # bank's has_written bits; writing one column of each
                        # accumulator slice orders it ahead of all 4 groups
                        nc.tensor.matmul(
                            avs[h][:, :, 0], idt, zrow,
                            start=True, stop=True, skip_group_check=True)
                    av = avs[h]
                    for t in range(QT):
                        nc.tensor.matmul(
                            av[:, t, :HD + 1],
                            et[:, h, t * P:(t + 1) * P],
                            vt[:, j, 2 * m + h, :],
                            start=False, stop=(j == SJ - 1 and t == QT - 1),
                            skip_group_check=True,
                        )

            # software pipeline: av(j-1) is emitted after score/exp(j) so the
            # in-order PE stream never stalls a score matmul behind an av
            # that waits on exp(j)
            pipe = []
            for j in range(SJ):
                et = att_exp(m, n, j, on_dve=dve_exp(n, j))
                if deferred and j in (2, 4, 6, 8):
                    deferred.pop(0)()
                pipe.append(et)
                if len(pipe) > 2:
                    emit_avs(pipe.pop(0), j - 2)
            emit_avs(pipe.pop(0), SJ - 2)
            emit_avs(pipe.pop(0), SJ - 1)
            for fn in deferred:
                fn()
            # norms/evictions ride DVE in the n=0 sweep and ScalarE in the
            # filler-dry n=1 sweep, where DVE carries half the exp load
            scr = scrp.tile([P, QT, P], dtype, tag="scr", name="scr")
            for h, av in enumerate(avs):
                rcp = scrp.tile([P, QT], F32, tag="rcp", name="rcp")
                nc.vector.reciprocal(rcp, av[:, :, HD])
                # one strided op for all 4 q-subtiles; the reciprocal rides
                # a stride-0 free dim
                nc.vector.tensor_tensor(
                    scr[:, :, h * HD:(h + 1) * HD],
                    av[:, :, :HD],
                    rcp.unsqueeze(-1).broadcast_to([P, QT, HD]),
                    ALU.mult)

            def transp(t):
                def emit():
                    tp = mmp.tile([P, P], dtype, tag="mm", name="tp")
                    nc.tensor.transpose(tp, scr[:, t, :], idt)
                    dst = aot[:, m, n * QC + t * P:n * QC + (t + 1) * P]
                    if n == 0:
                        nc.vector.tensor_copy(dst, tp)
                    else:
                        nc.scalar.copy(dst, tp)
                return emit

            return [transp(t) for t in range(QT)]

        def out_ln(t, mode="mid"):
            """Out-projection + LayerNorm for q row tile t. mode: "mid"
            keeps DVE light (it carries the exp offload), "spread" moves
            tail work onto the now-idle ScalarE/GpSimd, "final" minimizes
            the serial chain that sets the kernel end time."""
            FSUB = min(512, D)
            NSUB = D // FSUB
            ot = outp.tile([P, D], F32, tag="ot", name="ot")
            for nn in range(VN):
                ps = mmp.tile([P, 512], F32, tag="mm", name="ps")[:, :VC]
                for k in range(ET):
                    nc.tensor.matmul(
                        ps, aot[:, k, t * P:(t + 1) * P],
                        wo[:, k, nn * VC:(nn + 1) * VC],
                        start=(k == 0), stop=(k == ET - 1),
                    )
                if mode in ("final", "early_tail"):
                    # fused psum-read + bias add: shortest serial chain
                    nc.vector.tensor_tensor(
                        ot[:, nn * VC:(nn + 1) * VC], ps,
                        brow["bo"][:, nn * VC:(nn + 1) * VC], ALU.add)
                else:
                    # evict on ScalarE (Copy shares exp's table), bias-add
                    # on GpSimd: DVE stays free
                    nc.scalar.copy(ot[:, nn * VC:(nn + 1) * VC], ps)
                    nc.gpsimd.tensor_tensor(
                        ot[:, nn * VC:(nn + 1) * VC],
                        ot[:, nn * VC:(nn + 1) * VC],
                        brow["bo"][:, nn * VC:(nn + 1) * VC], ALU.add)
            scr = lnp.tile([P, NSUB * 6 + 6], F32, tag="scr", name="scr")
            stats = scr[:, 0:NSUB * 6].rearrange("p (s f) -> p s f", f=6)
            mv = scr[:, NSUB * 6:NSUB * 6 + 2]
            rstd = scr[:, NSUB * 6 + 2:NSUB * 6 + 3]
            sv = scr[:, NSUB * 6 + 3:NSUB * 6 + 4]
            nwt = scr[:, NSUB * 6 + 4:NSUB * 6 + 5]
            otv = ot.rearrange("p (s f) -> p s f", f=FSUB)
            for sbi in range(NSUB):
                nc.vector.bn_stats(stats[:, sbi, :], otv[:, sbi, :])
            nc.vector.bn_aggr(mv, stats)
            # rsqrt(var + eps) on DVE (quake seed + one Newton step) keeps
            # Sqrt off ScalarE: sqrt shares no activation table with exp, so
            # each interleaved use would cost two 1.3us table reloads
            I32 = mybir.dt.int32
            nc.vector.tensor_scalar(sv, mv[:, 1:2], 1e-5, None, ALU.add)
            nc.vector.tensor_scalar(rstd.bitcast(I32), sv.bitcast(I32),
                                    1, None, ALU.arith_shift_right)
            nc.vector.tensor_scalar(rstd.bitcast(I32), rstd.bitcast(I32),
                                    -1, 0x5F3759DF, ALU.mult, ALU.add)
            nc.vector.tensor_tensor(nwt, sv, rstd, ALU.mult)
            nc.vector.tensor_tensor(nwt, nwt, rstd, ALU.mult)
            nc.vector.tensor_scalar(nwt, nwt, -0.5, 1.5, ALU.mult, ALU.add)
            nc.vector.tensor_tensor(rstd, rstd, nwt, ALU.mult)
            if mode == "final":
                # normalize on ScalarE (drained of exps by now):
                # Identity(ot*rstd + (-mu*rstd)), both per-partition scalars
                nmu = scr[:, NSUB * 6 + 5:NSUB * 6 + 6]
                nc.vector.tensor_scalar(nmu, mv[:, 0:1], -1.0, None,
                                        ALU.mult)
                nc.vector.tensor_tensor(nmu, nmu, rstd, ALU.mult)
                nc.scalar.activation(ot, ot, AF.Identity, bias=nmu,
                                     scale=rstd)
            else:
                nc.vector.tensor_scalar(
                    ot, ot, mv[:, 0:1], rstd, ALU.subtract, ALU.mult)
            eng = nc.vector if mode == "final" else nc.gpsimd
            eng.tensor_tensor(ot, ot, brow["lnw"], ALU.mult)
            eng.tensor_tensor(ot, ot, brow["lnb"], ALU.add)
            nc.sync.dma_start(
                out.rearrange("(t p) d -> p t d", p=P)[:, t, :], ot)

        # --- emission schedule ---
        # q/k for the first v-chunk's head pairs, then v chunk 0, then
        # alternate attention blocks with the remaining projections so the
        # scheduler can fill PE gaps under ScalarE-bound attention.
        wo = wop.tile([P, ET, D], dtype, tag="wo")
        qk_proj(0, pre0)
        qk_proj(1, pre1)
        wv0 = load_wv(0)
        v_proj(0, wv0)
        nc.scalar.dma_start(wo, ws["wo"].rearrange("(t p) d -> p t d", p=P))
        # n-outer: the n=0 sweep over all head pairs interleaves with the
        # remaining q/k/v projections; after it, aot columns [0, QC) are
        # complete, so out_ln(0..QT-1) becomes the PE filler for the
        # otherwise projection-dry n=1 sweep.
        deferred = []
        for n in range(QN):
            if n == 1:
                q_late(0)
                q_late(1)
            for m in range(ET):
                deferred = attention(m, n, deferred)
                if n == 0:
                    if 2 <= m + 1 < ET and m + 1 != 1:
                        qk_proj(m + 1)
                    for vn in range(1, VN):
                        if m + 2 == vn * MPC:
                            v_proj(vn, load_wv(vn))
                else:
                    if m + 2 < ET:
                        q_late(m + 2)
                    if m - 4 >= 0:
                        out_ln(m - 4, mode="early_tail")
        for fn in deferred:
            fn()
        # tail: out-proj/LN for the last q-chunk's row tiles; the very
        # last tile gets the minimum-latency all-DVE chain
        for t in range(QT, TQ):
            out_ln(t, mode="final" if t == TQ - 1 else "early_tail")

    nc.compile()
    return nc


# ---------------------------------------------------------------- host side

_CACHE = {}


def _get_nc(S, SQ, D, H):
    key = (S, SQ, D, H)
    if key not in _CACHE:
        _CACHE[key] = build_bass(S, SQ, D, H)
    return _CACHE[key]


def make_in_maps(x, Wq, bq, Wk, bk, Wv, bv, Wo, bo, ln_w, ln_b, n_cores=8):
    """Shard full inputs into per-core input maps (batch x seq-half)."""
    B, S, D = x.shape
    halves = n_cores // B
    SQ = S // halves
    bf = ml_dtypes.bfloat16
    ET = D // 128

    def pack_qk(W):
        # [m, p, t*128+d] = W.T[t*128+p, m*128+d]
        w4 = np.asarray(W).T.reshape(ET, 128, ET, 128)  # [t, p, m, d]
        return np.ascontiguousarray(
            w4.transpose(2, 1, 0, 3).reshape(ET, 128, ET * 128)).astype(bf)

    common = {
        "ident": np.eye(128, dtype=bf),
        "wq": pack_qk(Wq),
        "wk": pack_qk(Wk),
        "wv": np.ascontiguousarray(np.asarray(Wv).T).astype(bf),
        "wo": np.ascontiguousarray(np.asarray(Wo).T).astype(bf),
        "bq": np.asarray(bq, np.float32), "bk": np.asarray(bk, np.float32),
        "bv": np.asarray(bv, np.float32), "bo": np.asarray(bo, np.float32),
        "lnw": np.asarray(ln_w, np.float32), "lnb": np.asarray(ln_b, np.float32),
    }
    in_maps = []
    for c in range(n_cores):
        b, half = c // halves, c % halves
        xTb = np.asarray(x[b]).T.astype(bf)  # [D, S]
        if half:
            xTb = np.roll(xTb, -half * SQ, axis=1)
        in_maps.append({"xT": np.ascontiguousarray(xTb), **common})
    return in_maps, SQ


def kernel(x, Wq, bq, Wk, bk, Wv, bv, Wo, bo, ln_w, ln_b, _trace=False):
    x = np.asarray(x)
    B, S, D = x.shape
    n_cores = 8
    in_maps, SQ = make_in_maps(x, Wq, bq, Wk, bk, Wv, bv, Wo, bo, ln_w, ln_b,
                               n_cores)
    nc = _get_nc(S, SQ, D, 16)
    res = run_bass_kernel_spmd(nc, in_maps, list(range(n_cores)), trace=_trace)
    out = np.empty((B, S, D), np.float32)
    halves = n_cores // B
    for c in range(n_cores):
        b, half = c // halves, c % halves
        out[b, half * SQ:(half + 1) * SQ] = res.results[c]["out"]
    kernel.last_result = res
    return out


if __name__ == "__main__":
    nc = build_bass(512, 256, 256, 4)
    print("built ok")



# revision 81
# speedup vs baseline: 1.0150x; 1.0013x over previous
"""Trainium2 Bass kernel for MultiHeadAttention (B=4, S=2048, D=1024, H=16).

Sharding: 8 cores = 4 batches x 2 sequence-halves, no collectives. Each
core computes full k/v projections for its batch and q/attention/
out-proj/LayerNorm for its half of the sequence. A host-side column
roll of x^T makes the program identical across cores (softmax over
keys is permutation-invariant, so the k/v column order doesn't
matter): the core's q rows are always columns [0, SQ) of its x^T.

Device program (per core), all matmuls bf16 with fp32 PSUM accumulate:
  qT = Wq @ xT[:, :SQ] + bq   [D, SQ]   (d on partitions)
  kT = Wk @ xT + bk           [D, S]
  v  = x @ Wv.T + bv          [S, D]    (s on partitions), augmented
                                        with a ones column per head
  attention per head-pair m (heads 2m / 2m+1 live on partition halves
  0:64 / 64:128 of kT/qT tile m), per q-chunk n:
    scT[j,h] = kT_h[:, j-chunk] . qT_h     per-head single-bank PSUM
                                           tiles (4-buf rotation) so the
                                           exp->score WAR pipelines at
                                           half-j granularity
    eT[j]    = exp(scT[j] / 8)             per-head ScalarE ops; half the
                                           j-tiles instead go through a
                                           DVE Schraudolph bit-trick
                                           (int16(A*s+B) viewed as bf16,
                                           ~1.8% rms on those keys)
    av_h[t] += eT_h[j][:, t-chunk].T @ v_aug_h[j]
        "AV-swap": eT is the stationary operand, v_aug (64 cols + ones
        column) moves, so each matmul streams only 65 columns instead
        of 512 -> half the PE time of the classic v.T @ e orientation.
        Accumulators for one head live in one PSUM bank as [4, 65]
        slices of a [128, 4, 128] tile; a tiny full-bank zero matmul
        opens each round (clears the has_written bits + orders the
        start), then all real matmuls accumulate with start=False.
    ao_h[t]  = av_h[t][:, 0:64] * (1/av_h[t][:, 64])   per-partition
                                           scalar -> one strided DVE op
    aot      = PE-transpose(ao)            [q, d] -> [d, q] via the
                                           identity matmul, per 128x128
  out = aoT.T @ Wo.T + bo; LayerNorm -> DRAM (rsqrt via DVE quake
  seed + Newton so Sqrt never evicts ScalarE's exp table).

Scheduling: n-outer sweeps; the n=0 sweep interleaves with the k/v and
n=0 q projections, the n=1 sweep is fed by the deferred n=1 q chunks
and the first half's out-proj/LayerNorm tiles. av(j-1) is emitted
after score/exp(j) and each block's transposes are woven into the next
block's j-loop so the in-order PE stream never head-of-line blocks.
"""

import os
import sys
from contextlib import ExitStack

for _p in ("/opt/trn_rl_repo", "/root/.axon_site/_ro/trn_rl_repo"):
    if _p not in sys.path and os.path.isdir(_p):
        sys.path.insert(0, _p)

# The kernel executes through the axon jax platform; a cpu-pinned
# JAX_PLATFORMS (used for running references) would hide the NeuronCores.
# Only safe to fix before jax is first imported.
if "jax" not in sys.modules and "axon" not in os.environ.get(
        "JAX_PLATFORMS", "axon"):
    os.environ.pop("JAX_PLATFORMS")

import ml_dtypes
import numpy as np

import concourse.bacc as bacc
import concourse.mybir as mybir
import concourse.tile as tile
from concourse import library_config
from concourse.bass_utils import run_bass_kernel_spmd

BF16 = mybir.dt.bfloat16
F32 = mybir.dt.float32
AF = mybir.ActivationFunctionType
ALU = mybir.AluOpType

HD = 64  # head dim


def build_bass(S, SQ, D, H, dtype=BF16):
    """Build the per-core Bass program. S: kv seq len, SQ: q rows handled
    by this core, D: embed dim, H: total heads."""
    assert D == H * HD
    P = 128
    ET = D // P           # e (contraction) tiles; also head-pair count
    QC = min(512, SQ)     # q free-dim chunk
    QN = SQ // QC
    KC = min(512, S)      # k-proj free-dim chunk
    KN = S // KC
    VC = min(512, D)      # v/out-proj d chunk
    VN = D // VC
    HPC = VC // HD        # heads per v chunk
    MPC = HPC // 2        # head-pairs per v chunk
    SJ = S // P           # key tiles
    TQ = SQ // P          # q row tiles

    nc = bacc.Bacc("TRN2", debug=False)

    xT = nc.dram_tensor("xT", [D, S], dtype, kind="ExternalInput").ap()
    ident = nc.dram_tensor("ident", [P, P], dtype, kind="ExternalInput").ap()
    ws = {}
    for w in ("wq", "wk"):  # host-packed per head-pair: [m, p, t*d]
        ws[w] = nc.dram_tensor(w, [ET, P, ET * P], dtype,
                               kind="ExternalInput").ap()
    for w in ("wv", "wo"):
        ws[w] = nc.dram_tensor(w, [D, D], dtype, kind="ExternalInput").ap()
    bs = {
        b: nc.dram_tensor(b, [D], F32, kind="ExternalInput").ap()
        for b in ("bq", "bk", "bv", "bo", "lnw", "lnb")
    }
    out = nc.dram_tensor("out", [SQ, D], F32, kind="ExternalOutput").ap()

    with tile.TileContext(nc) as tc, ExitStack() as ctx:
        singles = ctx.enter_context(tc.tile_pool(name="singles", bufs=1))
        qkv = ctx.enter_context(tc.tile_pool(name="qkv", bufs=1))
        xp = ctx.enter_context(tc.tile_pool(name="xp", bufs=1))
        wqk = ctx.enter_context(tc.tile_pool(name="wqk", bufs=3))
        wvp = ctx.enter_context(tc.tile_pool(name="wvp", bufs=1))
        wop = ctx.enter_context(tc.tile_pool(name="wop", bufs=1))
        expp = ctx.enter_context(tc.tile_pool(name="expp", bufs=6))
        scrp = ctx.enter_context(tc.tile_pool(name="scrp", bufs=6))
        outp = ctx.enter_context(tc.tile_pool(name="outp", bufs=3))
        lnp = ctx.enter_context(tc.tile_pool(name="lnp", bufs=3))
        mmp = ctx.enter_context(tc.tile_pool(name="mm", bufs=2, space="PSUM"))
        scp = ctx.enter_context(tc.tile_pool(name="scp", bufs=4, space="PSUM"))
        avp = ctx.enter_context(tc.tile_pool(name="avp", bufs=2, space="PSUM"))

        nc.gpsimd.load_library(library_config.proxy)

        qt = qkv.tile([P, ET, SQ], dtype, tag="qt")
        kt = qkv.tile([P, ET, S], dtype, tag="kt")
        vt = qkv.tile([P, SJ, H, HD + 1], dtype, tag="vt")
        aot = qkv.tile([P, ET, SQ], dtype, tag="aot")

        def load_wqk(m):
            wqm = wqk.tile([P, ET, P], dtype, tag="wqk", name="wqm")
            nc.scalar.dma_start(wqm, ws["wq"][m].rearrange("p (t d) -> p t d", d=P))
            wkm = wqk.tile([P, ET, P], dtype, tag="wqk", name="wkm")
            nc.scalar.dma_start(wkm, ws["wk"][m].rearrange("p (t d) -> p t d", d=P))
            return wqm, wkm

        # --- startup-ordered DMA: only what the first q-proj group and
        # first scores need goes ahead of the xt half-0 tiles on each
        # queue; the remaining weights follow behind
        def load_w(which, m, name):
            w = wqk.tile([P, ET, P], dtype, tag="wqk", name=name)
            nc.scalar.dma_start(
                w, ws[which][m].rearrange("p (t d) -> p t d", d=P))
            return w

        bqk = singles.tile([P, 2 * ET], F32, tag="bqk")
        nc.sync.dma_start(bqk[:, :ET], bs["bq"].rearrange("(t p) -> p t", p=P))
        wq0 = load_w("wq", 0, "wq0")
        xt = xp.tile([P, ET, S], dtype, tag="xt")
        H2 = S // 2
        xt_engs = (nc.sync, nc.scalar)
        xrow = xT.rearrange("(t p) s -> p t s", p=P)
        for k in range(ET):
            xt_engs[k % 2].dma_start(xt[:, k, 0:H2], xrow[:, k, 0:H2])
        wk0 = load_w("wk", 0, "wk0")
        pre0 = (wq0, wk0)
        pre1 = (load_w("wq", 1, "wq1"), load_w("wk", 1, "wk1"))
        for k in range(ET):
            xt_engs[k % 2].dma_start(xt[:, k, H2:S], xrow[:, k, H2:S])

        # --- constants ---
        nc.sync.dma_start(bqk[:, ET:], bs["bk"].rearrange("(t p) -> p t", p=P))
        # free-dim bias rows, physically replicated across partitions
        # (compute engines can't read partition-step-0 APs)
        brow = {}
        for b in ("bv", "bo", "lnw", "lnb"):
            t = singles.tile([P, D], F32, tag=b)
            nc.sync.dma_start(t[0:1, :], bs[b][None, :])
            nc.gpsimd.partition_broadcast(t, t[0:1, :])
            brow[b] = t
        eps = singles.tile([P, 1], F32, tag="eps")
        nc.vector.memset(eps, 1e-5)
        nc.vector.memset(vt[:, :, :, HD:HD + 1], 1.0)
        idt = singles.tile([P, P], dtype, tag="idt")
        nc.sync.dma_start(idt, ident)
        zrow = singles.tile([P, 4], dtype, tag="zrow")
        nc.vector.memset(zrow, 0.0)

        def q_chunk(m, wqm, n):
            ps = mmp.tile([P, 512], F32, tag="mm", name="ps")[:, :QC]
            for k in range(ET):
                nc.tensor.matmul(
                    ps, wqm[:, k, :], xt[:, k, n * QC:(n + 1) * QC],
                    start=(k == 0), stop=(k == ET - 1),
                )
            nc.vector.tensor_scalar_add(
                qt[:, m, n * QC:(n + 1) * QC], ps, bqk[:, m:m + 1])

        def q_late(m):
            """q chunks beyond n=0, deferred into the n=1 sweep as PE
            filler (the wq tile is re-fetched; DMA has big slack)."""
            wqm = wqk.tile([P, ET, P], dtype, tag="wqk", name="wqm2")
            nc.scalar.dma_start(
                wqm, ws["wq"][m].rearrange("p (t d) -> p t d", d=P))
            for n in range(1, QN):
                ps = mmp.tile([P, 512], F32, tag="mm", name="ps")[:, :QC]
                for k in range(ET):
                    nc.tensor.matmul(
                        ps, wqm[:, k, :], xt[:, k, n * QC:(n + 1) * QC],
                        start=(k == 0), stop=(k == ET - 1),
                    )
                nc.scalar.activation(
                    qt[:, m, n * QC:(n + 1) * QC], ps, AF.Identity,
                    bias=bqk[:, m:m + 1])

        def qk_proj(m, pre=None):
            """q (n=0 chunk) and k projections for head-pair m."""
            wqm, wkm = pre if pre is not None else load_wqk(m)
            q_chunk(m, wqm, 0)
            for n in range(KN):
                ps = mmp.tile([P, 512], F32, tag="mm", name="ps")[:, :KC]
                for k in range(ET):
                    nc.tensor.matmul(
                        ps, wkm[:, k, :], xt[:, k, n * KC:(n + 1) * KC],
                        start=(k == 0), stop=(k == ET - 1),
                    )
                nc.vector.tensor_scalar_add(
                    kt[:, m, n * KC:(n + 1) * KC], ps,
                    bqk[:, ET + m:ET + m + 1])

        def load_wv(n):
            wvn = wvp.tile([P, ET, VC], dtype, tag="wv", name="wvn")
            nc.scalar.dma_start(
                wvn,
                ws["wv"].rearrange("(t p) d -> p t d", p=P)[:, :, n * VC:(n + 1) * VC])
            return wvn

        def v_block(n, wvn, j0):
            """v projection d-chunk n, s-tiles j0..j0+1."""
            for j in range(j0, min(j0 + 2, SJ)):
                ps = mmp.tile([P, 512], F32, tag="mm", name="ps")[:, :VC]
                for k in range(ET):
                    nc.tensor.matmul(
                        ps, xt[:, k, j * P:(j + 1) * P], wvn[:, k, :],
                        start=(k == 0), stop=(k == ET - 1),
                    )
                nc.vector.tensor_tensor(
                    vt[:, j, n * HPC:(n + 1) * HPC, 0:HD],
                    ps.rearrange("p (h d) -> p h d", d=HD),
                    brow["bv"][:, n * VC:(n + 1) * VC].rearrange(
                        "p (h d) -> p h d", d=HD),
                    ALU.add,
                )

        def v_proj(n, wvn):
            for j0 in range(0, SJ, 2):
                v_block(n, wvn, j0)

        # Schraudolph exp for the DVE-offloaded score tiles: the bf16 bit
        # pattern of exp(s/8) is approximated by int16(s*A + B); the linear
        # mantissa interpolation costs ~1.8% rms on those keys, which the
        # softmax ratio mostly cancels.
        SCH_A = 0.125 * 128.0 / float(np.log(2.0))
        SCH_B = 16256.0 - 7.25

        def dve_exp(n, j):
            # the n=0 sweep is PE-bound (projection filler available), so
            # ScalarE alone keeps up; the filler-dry n=1 sweep needs the
            # exp cadence split across both engines
            return (j % 2 == 1) != (j >= 14)

        def att_exp(m, n, j, on_dve=False):
            """score pair + exp for (head pair m, q-chunk n, k-tile j).

            Per-head single-bank score tiles: exp of head h waits only on
            head h's score matmul, and the next scores' WAR on the psum
            bank clears per head — a 4-deep half-j pipeline instead of a
            2-deep full-j one."""
            et = expp.tile([P, 2, 512], dtype, tag="exp", name="et")
            for h in range(2):
                sc = scp.tile([P, 512], F32, tag="sc", name="sc")
                nc.tensor.matmul(
                    sc[:, :QC],
                    kt[h * HD:(h + 1) * HD, m, j * P:(j + 1) * P],
                    qt[h * HD:(h + 1) * HD, m, n * QC:(n + 1) * QC],
                )
                if on_dve:
                    nc.vector.tensor_scalar(
                        et.bitcast(mybir.dt.int16)[:, h, :QC], sc[:, :QC],
                        SCH_A, SCH_B, ALU.mult, ALU.add)
                else:
                    nc.scalar.activation(et[:, h, :QC], sc[:, :QC],
                                         AF.Exp, scale=0.125)
            return et

        QT = QC // P  # 128-row q subtiles per q-chunk

        def attention(m, n, deferred):
            """q-chunk n of head pair m (heads 2m, 2m+1). AV-swap: et is
            stationary, v_aug moves; accumulators av_h[:, t, 0:65] hold
            [q, av|denom] per q-subtile t, one PSUM bank per head.

            `deferred` holds the previous block's transpose emitters; they
            are woven into this block's j-loop so the in-order PE stream
            never stalls on the previous block's DVE norms at the block
            boundary. Returns this block's deferred emitters."""
            avs = [None, None]

            def emit_avs(et, j):
                for h in range(2):
                    if avs[h] is None:
                        # lazy: the zero matmul (clears the bank's
                        # has_written bits and orders, via the full-tile
                        # write, ahead of the start=False accumulation)
                        # waits on the previous block's norm reads through
                        # pool-buf reuse, so it must come after this
                        # block's first scores in the stream
                        avs[h] = avp.tile([P, QT, P], F32, tag="av",
                                          name="av")
                        # 4-element zero matmul: start=True clears the whole
                        # bank's has_written bits; writing one column of each
                        # accumulator slice orders it ahead of all 4 groups
                        nc.tensor.matmul(
                            avs[h][:, :, 0], idt, zrow,
                            start=True, stop=True, skip_group_check=True)
                    av = avs[h]
                    for t in range(QT):
                        nc.tensor.matmul(
                            av[:, t, :HD + 1],
                            et[:, h, t * P:(t + 1) * P],
                            vt[:, j, 2 * m + h, :],
                            start=False, stop=(j == SJ - 1 and t == QT - 1),
                            skip_group_check=True,
                        )

            # software pipeline: av(j-1) is emitted after score/exp(j) so the
            # in-order PE stream never stalls a score matmul behind an av
            # that waits on exp(j)
            pipe = []
            for j in range(SJ):
                et = att_exp(m, n, j, on_dve=dve_exp(n, j))
                if deferred and j in (2, 4, 6, 8):
                    deferred.pop(0)()
                pipe.append(et)
                if len(pipe) > 2:
                    emit_avs(pipe.pop(0), j - 2)
            emit_avs(pipe.pop(0), SJ - 2)
            emit_avs(pipe.pop(0), SJ - 1)
            for fn in deferred:
                fn()
            # norms/evictions ride DVE in the n=0 sweep and ScalarE in the
            # filler-dry n=1 sweep, where DVE carries half the exp load
            scr = scrp.tile([P, QT, P], dtype, tag="scr", name="scr")
            for h, av in enumerate(avs):
                rcp = scrp.tile([P, QT], F32, tag="rcp", name="rcp")
                nc.vector.reciprocal(rcp, av[:, :, HD])
                # one strided op for all 4 q-subtiles; the reciprocal rides
                # a stride-0 free dim
                nc.vector.tensor_tensor(
                    scr[:, :, h * HD:(h + 1) * HD],
                    av[:, :, :HD],
                    rcp.unsqueeze(-1).broadcast_to([P, QT, HD]),
                    ALU.mult)

            def transp(t):
                def emit():
                    tp = mmp.tile([P, P], dtype, tag="mm", name="tp")
                    nc.tensor.transpose(tp, scr[:, t, :], idt)
                    dst = aot[:, m, n * QC + t * P:n * QC + (t + 1) * P]
                    if n == 0:
                        nc.vector.tensor_copy(dst, tp)
                    else:
                        nc.scalar.copy(dst, tp)
                return emit

            return [transp(t) for t in range(QT)]

        def out_ln(t, mode="mid"):
            """Out-projection + LayerNorm for q row tile t. mode: "mid"
            keeps DVE light (it carries the exp offload), "spread" moves
            tail work onto the now-idle ScalarE/GpSimd, "final" minimizes
            the serial chain that sets the kernel end time."""
            FSUB = min(512, D)
            NSUB = D // FSUB
            ot = outp.tile([P, D], F32, tag="ot", name="ot")
            for nn in range(VN):
                ps = mmp.tile([P, 512], F32, tag="mm", name="ps")[:, :VC]
                for k in range(ET):
                    nc.tensor.matmul(
                        ps, aot[:, k, t * P:(t + 1) * P],
                        wo[:, k, nn * VC:(nn + 1) * VC],
                        start=(k == 0), stop=(k == ET - 1),
                    )
                if mode in ("final", "early_tail"):
                    # fused psum-read + bias add: shortest serial chain
                    nc.vector.tensor_tensor(
                        ot[:, nn * VC:(nn + 1) * VC], ps,
                        brow["bo"][:, nn * VC:(nn + 1) * VC], ALU.add)
                else:
                    # evict on ScalarE (Copy shares exp's table), bias-add
                    # on GpSimd: DVE stays free
                    nc.scalar.copy(ot[:, nn * VC:(nn + 1) * VC], ps)
                    nc.gpsimd.tensor_tensor(
                        ot[:, nn * VC:(nn + 1) * VC],
                        ot[:, nn * VC:(nn + 1) * VC],
                        brow["bo"][:, nn * VC:(nn + 1) * VC], ALU.add)
            scr = lnp.tile([P, NSUB * 6 + 5], F32, tag="scr", name="scr")
            stats = scr[:, 0:NSUB * 6].rearrange("p (s f) -> p s f", f=6)
            mv = scr[:, NSUB * 6:NSUB * 6 + 2]
            rstd = scr[:, NSUB * 6 + 2:NSUB * 6 + 3]
            sv = scr[:, NSUB * 6 + 3:NSUB * 6 + 4]
            nwt = scr[:, NSUB * 6 + 4:NSUB * 6 + 5]
            otv = ot.rearrange("p (s f) -> p s f", f=FSUB)
            for sbi in range(NSUB):
                nc.vector.bn_stats(stats[:, sbi, :], otv[:, sbi, :])
            nc.vector.bn_aggr(mv, stats)
            # rsqrt(var + eps) on DVE (quake seed + one Newton step) keeps
            # Sqrt off ScalarE: sqrt shares no activation table with exp, so
            # each interleaved use would cost two 1.3us table reloads
            I32 = mybir.dt.int32
            nc.vector.tensor_scalar(sv, mv[:, 1:2], 1e-5, None, ALU.add)
            nc.vector.tensor_scalar(rstd.bitcast(I32), sv.bitcast(I32),
                                    1, None, ALU.arith_shift_right)
            nc.vector.tensor_scalar(rstd.bitcast(I32), rstd.bitcast(I32),
                                    -1, 0x5F3759DF, ALU.mult, ALU.add)
            nc.vector.tensor_tensor(nwt, sv, rstd, ALU.mult)
            nc.vector.tensor_tensor(nwt, nwt, rstd, ALU.mult)
            nc.vector.tensor_scalar(nwt, nwt, -0.5, 1.5, ALU.mult, ALU.add)
            nc.vector.tensor_tensor(rstd, rstd, nwt, ALU.mult)
            nc.vector.tensor_scalar(
                ot, ot, mv[:, 0:1], rstd, ALU.subtract, ALU.mult)
            eng = nc.vector if mode == "final" else nc.gpsimd
            eng.tensor_tensor(ot, ot, brow["lnw"], ALU.mult)
            eng.tensor_tensor(ot, ot, brow["lnb"], ALU.add)
            nc.sync.dma_start(
                out.rearrange("(t p) d -> p t d", p=P)[:, t, :], ot)

        # --- emission schedule ---
        # q/k for the first v-chunk's head pairs, then v chunk 0, then
        # alternate attention blocks with the remaining projections so the
        # scheduler can fill PE gaps under ScalarE-bound attention.
        wo = wop.tile([P, ET, D], dtype, tag="wo")
        qk_proj(0, pre0)
        qk_proj(1, pre1)
        wv0 = load_wv(0)
        v_proj(0, wv0)
        nc.scalar.dma_start(wo, ws["wo"].rearrange("(t p) d -> p t d", p=P))
        # n-outer: the n=0 sweep over all head pairs interleaves with the
        # remaining q/k/v projections; after it, aot columns [0, QC) are
        # complete, so out_ln(0..QT-1) becomes the PE filler for the
        # otherwise projection-dry n=1 sweep.
        deferred = []
        for n in range(QN):
            if n == 1:
                q_late(0)
                q_late(1)
            for m in range(ET):
                deferred = attention(m, n, deferred)
                if n == 0:
                    if 2 <= m + 1 < ET and m + 1 != 1:
                        qk_proj(m + 1)
                    for vn in range(1, VN):
                        if m + 2 == vn * MPC:
                            v_proj(vn, load_wv(vn))
                else:
                    if m + 2 < ET:
                        q_late(m + 2)
                    if m - 4 >= 0:
                        out_ln(m - 4, mode="early_tail")
        for fn in deferred:
            fn()
        # tail: out-proj/LN for the last q-chunk's row tiles; the very
        # last tile gets the minimum-latency all-DVE chain
        for t in range(QT, TQ):
            out_ln(t, mode="final" if t == TQ - 1 else "early_tail")

    nc.compile()
    return nc


# ---------------------------------------------------------------- host side

_CACHE = {}


def _get_nc(S, SQ, D, H):
    key = (S, SQ, D, H)
    if key not in _CACHE:
        _CACHE[key] = build_bass(S, SQ, D, H)
    return _CACHE[key]


def make_in_maps(x, Wq, bq, Wk, bk, Wv, bv, Wo, bo, ln_w, ln_b, n_cores=8):
    """Shard full inputs into per-core input maps (batch x seq-half)."""
    B, S, D = x.shape
    halves = n_cores // B
    SQ = S // halves
    bf = ml_dtypes.bfloat16
    ET = D // 128

    def pack_qk(W):
        # [m, p, t*128+d] = W.T[t*128+p, m*128+d]
        w4 = np.asarray(W).T.reshape(ET, 128, ET, 128)  # [t, p, m, d]
        return np.ascontiguousarray(
            w4.transpose(2, 1, 0, 3).reshape(ET, 128, ET * 128)).astype(bf)

    common = {
        "ident": np.eye(128, dtype=bf),
        "wq": pack_qk(Wq),
        "wk": pack_qk(Wk),
        "wv": np.ascontiguousarray(np.asarray(Wv).T).astype(bf),
        "wo": np.ascontiguousarray(np.asarray(Wo).T).astype(bf),
        "bq": np.asarray(bq, np.float32), "bk": np.asarray(bk, np.float32),
        "bv": np.asarray(bv, np.float32), "bo": np.asarray(bo, np.float32),
        "lnw": np.asarray(ln_w, np.float32), "lnb": np.asarray(ln_b, np.float32),
    }
    in_maps = []
    for c in range(n_cores):
        b, half = c // halves, c % halves
        xTb = np.asarray(x[b]).T.astype(bf)  # [D, S]
        if half:
            xTb = np.roll(xTb, -half * SQ, axis=1)
        in_maps.append({"xT": np.ascontiguousarray(xTb), **common})
    return in_maps, SQ


def kernel(x, Wq, bq, Wk, bk, Wv, bv, Wo, bo, ln_w, ln_b, _trace=False):
    x = np.asarray(x)
    B, S, D = x.shape
    n_cores = 8
    in_maps, SQ = make_in_maps(x, Wq, bq, Wk, bk, Wv, bv, Wo, bo, ln_w, ln_b,
                               n_cores)
    nc = _get_nc(S, SQ, D, 16)
    res = run_bass_kernel_spmd(nc, in_maps, list(range(n_cores)), trace=_trace)
    out = np.empty((B, S, D), np.float32)
    halves = n_cores // B
    for c in range(n_cores):
        b, half = c // halves, c % halves
        out[b, half * SQ:(half + 1) * SQ] = res.results[c]["out"]
    kernel.last_result = res
    return out


if __name__ == "__main__":
    nc = build_bass(512, 256, 256, 4)
    print("built ok")

